# revision 6
# baseline (speedup 1.0000x reference)
"""Trainium2 Bass kernel for nn_MultiHeadAttention_26482768347194.

Key algebraic fact: the reference applies softmax over a size-1 trailing
axis, so the attention score matrix is exactly all-ones.  The whole module
collapses (exactly, in real arithmetic) to

    xsum[b]   = sum_l x[b, l, :]                        # (D,)
    t[b]      = xsum[b] @ wv + L * bv                   # (H*D,)
    z[b]      = t[b] @ fc_w + fc_b                      # (D,)
    y[b,l,:]  = x[b,l,:] + z[b]
    out       = LayerNorm(y) * ln_g + ln_b              # over last dim

q/k/tanh/score inputs are mathematically dead.

Sharding: data-parallel over batch, one batch element per NeuronCore
(B == 8 == n_cores).  Each core reads its x shard, computes z for its
batch with the (replicated) weight matrices on the TensorEngine, then
does the residual + LayerNorm on Vector/Scalar engines.

This file is self-contained: shapes are hardcoded, no sibling imports.
"""

import os
from contextlib import ExitStack

import numpy as np

import concourse.bass as bass
import concourse.bacc as bacc
import concourse.mybir as mybir
import concourse.tile as tile
from concourse.bass_utils import run_bass_kernel_spmd

B, L, D, H = 8, 1024, 512, 8
HD = H * D          # 4096
P = 128             # partitions
NT = L // P         # 8 token tiles per core
KD = D // P         # 4 contraction chunks over d
KO = HD // P        # 32 contraction chunks over h*d
EPS = 1e-5
N_CORES = 8

F32 = mybir.dt.float32
AF = mybir.ActivationFunctionType
ALU = mybir.AluOpType


def build_kernel():
    nc = bacc.Bacc("TRN2", target_bir_lowering=False, debug=False,
                   num_devices=N_CORES)

    x_d = nc.dram_tensor("x", [L, D], F32, kind="ExternalInput")
    wv_d = nc.dram_tensor("wv", [D, HD], F32, kind="ExternalInput")
    bvT_d = nc.dram_tensor("bvT", [P, KO], F32, kind="ExternalInput")
    fc_d = nc.dram_tensor("fc_w", [HD, D], F32, kind="ExternalInput")
    fcb_d = nc.dram_tensor("fc_b", [1, D], F32, kind="ExternalInput")
    g_d = nc.dram_tensor("ln_g", [1, D], F32, kind="ExternalInput")
    b_d = nc.dram_tensor("ln_b", [1, D], F32, kind="ExternalInput")
    out_d = nc.dram_tensor("out", [L, D], F32, kind="ExternalOutput")

    x_v = x_d.ap().rearrange("(t p) d -> t p d", p=P)      # [NT, P, D]
    wv_v = wv_d.ap().rearrange("(k p) n -> k p n", p=P)    # [KD, P, HD]
    fc_v = fc_d.ap().rearrange("(o p) d -> o p d", p=P)    # [KO, P, D]
    out_v = out_d.ap().rearrange("(t p) d -> t p d", p=P)  # [NT, P, D]

    with tile.TileContext(nc) as tc, ExitStack() as ctx:
        consts = ctx.enter_context(tc.tile_pool(name="consts", bufs=1))
        xp = ctx.enter_context(tc.tile_pool(name="xp", bufs=1))
        wvp = ctx.enter_context(tc.tile_pool(name="wvp", bufs=2))
        fcp = ctx.enter_context(tc.tile_pool(name="fcp", bufs=1))
        work = ctx.enter_context(tc.tile_pool(name="work", bufs=3))
        psum = ctx.enter_context(
            tc.tile_pool(name="psum", bufs=1, space=bass.MemorySpace.PSUM))
        psum4 = ctx.enter_context(
            tc.tile_pool(name="psum4", bufs=4, space=bass.MemorySpace.PSUM))

        # ---- constants -------------------------------------------------
        ones_col = consts.tile([P, 1], F32)      # matmul rhs for token sums
        nc.gpsimd.memset(ones_col[:], 1.0)
        ones_row = consts.tile([1, P], F32)      # matmul lhsT for broadcasts
        nc.gpsimd.memset(ones_row[:], 1.0)
        eps_t = consts.tile([P, 1], F32)         # layernorm epsilon
        nc.gpsimd.memset(eps_t[:], EPS)

        bvT_t = consts.tile([P, KO], F32)
        nc.sync.dma_start(bvT_t[:], bvT_d.ap())
        bv1024 = consts.tile([P, KO], F32)
        nc.scalar.mul(bv1024[:], bvT_t[:], float(L))

        fcb_t = consts.tile([1, D], F32)
        nc.sync.dma_start(fcb_t[:], fcb_d.ap())
        g_t = consts.tile([1, D], F32)
        nc.sync.dma_start(g_t[:], g_d.ap())
        b_t = consts.tile([1, D], F32)
        nc.sync.dma_start(b_t[:], b_d.ap())

        # broadcast ln_g / ln_b to all 128 partitions via a K=1 matmul
        g_bc = consts.tile([P, D], F32)
        b_bc = consts.tile([P, D], F32)
        for src, dst in ((g_t, g_bc), (b_t, b_bc)):
            pb = psum.tile([P, D], F32, tag="bcast")
            nc.tensor.matmul(pb[:], ones_row[:], src[:], start=True, stop=True)
            nc.vector.tensor_copy(dst[:], pb[:])

        # ---- load x (stays resident in SBUF) ---------------------------
        x_t = xp.tile([P, NT, D], F32)
        for t in range(NT):
            nc.sync.dma_start(x_t[:, t, :], x_v[t])

        # ---- phase A: xsumT[d] = sum over tokens of x -------------------
        # lhsT = x tile slice [128 tok, 128 d], rhs = ones -> psum [128 d, 1]
        ps_xs = psum.tile([P, KD], F32, tag="xs")
        for c in range(KD):
            for t in range(NT):
                nc.tensor.matmul(
                    ps_xs[:, c:c + 1],
                    x_t[:, t, c * P:(c + 1) * P],
                    ones_col[:],
                    start=(t == 0), stop=(t == NT - 1))
        xsumT = consts.tile([P, KD], F32)
        nc.vector.tensor_copy(xsumT[:], ps_xs[:])

        # ---- phase B: tT = wv.T @ xsumT  (t = xsum @ wv) ---------------
        # stationary = wv chunk [128 d, 128 hd], moving = xsumT column
        ps_tk = []
        for kd in range(KD):
            wv_t = wvp.tile([P, HD], F32, tag="wv")
            for j in range(4):
                sl = slice(j * (HD // 4), (j + 1) * (HD // 4))
                nc.sync.dma_start(wv_t[:, sl], wv_v[kd][:, sl])
            pt = psum4.tile([P, KO], F32, tag="tk")
            for o in range(KO):
                nc.tensor.matmul(
                    pt[:, o:o + 1],
                    wv_t[:, o * P:(o + 1) * P],
                    xsumT[:, kd:kd + 1],
                    start=True, stop=True)
            ps_tk.append(pt)

        tT = consts.tile([P, KO], F32)
        nc.vector.tensor_add(tT[:], bv1024[:], ps_tk[0][:])
        for kd in range(1, KD):
            nc.vector.tensor_add(tT[:], tT[:], ps_tk[kd][:])

        # ---- phase C: z = t @ fc_w  ------------------------------------
        # stationary = tT column [128 hd, 1], moving = fc_w rows [128 hd, D]
        fc_tiles = []
        for gidx in range(KO // 4):
            ft = fcp.tile([P, 4, D], F32, tag=f"fc{gidx}")
            for q in range(4):
                nc.sync.dma_start(ft[:, q, :], fc_v[gidx * 4 + q])
            fc_tiles.append(ft)

        ps_z = psum.tile([1, D], F32, tag="z")
        for o in range(KO):
            nc.tensor.matmul(
                ps_z[:],
                tT[:, o:o + 1],
                fc_tiles[o // 4][:, o % 4, :],
                start=(o == 0), stop=(o == KO - 1))

        z_sb = consts.tile([1, D], F32)
        nc.vector.tensor_add(z_sb[:], ps_z[:], fcb_t[:])

        # broadcast z to all partitions
        zb = consts.tile([P, D], F32)
        ps_zb = psum.tile([P, D], F32, tag="bcast")
        nc.tensor.matmul(ps_zb[:], ones_row[:], z_sb[:], start=True, stop=True)
        nc.vector.tensor_copy(zb[:], ps_zb[:])

        # ---- phase D: residual + layernorm per token tile --------------
        for t in range(NT):
            y = work.tile([P, D], F32, tag="y")
            nc.vector.tensor_add(y[:], x_t[:, t, :], zb[:])

            s6 = work.tile([P, 6], F32, tag="s6")
            nc.vector.bn_stats(s6[:], y[:])
            mv = work.tile([P, 2], F32, tag="mv")
            nc.vector.bn_aggr(mv[:], s6[:])

            std = work.tile([P, 1], F32, tag="std")
            nc.scalar.activation(std[:], mv[:, 1:2], AF.Sqrt, bias=eps_t[:])
            rstd = work.tile([P, 1], F32, tag="rstd")
            nc.vector.reciprocal(rstd[:], std[:])
            nb = work.tile([P, 1], F32, tag="nb")
            nc.vector.scalar_tensor_tensor(
                nb[:], mv[:, 0:1], -1.0, rstd[:], op0=ALU.mult, op1=ALU.mult)

            xn = work.tile([P, D], F32, tag="xn")
            nc.scalar.activation(xn[:], y[:], AF.Identity,
                                 bias=nb[:], scale=rstd[:])

            o1 = work.tile([P, D], F32, tag="o1")
            nc.vector.tensor_mul(o1[:], xn[:], g_bc[:])
            o2 = work.tile([P, D], F32, tag="o2")
            nc.vector.tensor_add(o2[:], o1[:], b_bc[:])

            nc.sync.dma_start(out_v[t], o2[:])

    nc.compile()
    return nc


_NC_CACHE = None


def _get_nc():
    global _NC_CACHE
    if _NC_CACHE is None:
        _NC_CACHE = build_kernel()
    return _NC_CACHE


def _shard_inputs(inputs):
    x = np.ascontiguousarray(np.asarray(inputs["input"], dtype=np.float32))
    wv = np.ascontiguousarray(np.asarray(inputs["wv"], dtype=np.float32))
    bv = np.asarray(inputs["bv"], dtype=np.float32)
    fc_w = np.ascontiguousarray(np.asarray(inputs["fc_w"], dtype=np.float32))
    fc_b = np.asarray(inputs["fc_b"], dtype=np.float32)
    ln_g = np.asarray(inputs["ln_g"], dtype=np.float32)
    ln_b = np.asarray(inputs["ln_b"], dtype=np.float32)

    bvT = np.ascontiguousarray(bv.reshape(KO, P).T)          # [128, 32]
    fcb = np.ascontiguousarray(fc_b[None, :])
    g = np.ascontiguousarray(ln_g[None, :])
    b = np.ascontiguousarray(ln_b[None, :])

    in_maps = []
    for i in range(N_CORES):
        in_maps.append({
            "x": np.ascontiguousarray(x[i]),
            "wv": wv,
            "bvT": bvT,
            "fc_w": fc_w,
            "fc_b": fcb,
            "ln_g": g,
            "ln_b": b,
        })
    return in_maps


def kernel(**inputs) -> np.ndarray:
    nc = _get_nc()
    in_maps = _shard_inputs(inputs)
    res = run_bass_kernel_spmd(nc, in_maps, core_ids=list(range(N_CORES)))
    out = np.stack([res.results[i]["out"] for i in range(N_CORES)], axis=0)
    return out.astype(np.float32)


def _install_ntff_hook_shim():
    """Bridge trn_boot's ctypes NTFF profiler into antenv.axon_hooks,
    which bass_utils imports when trace=True under axon."""
    import sys
    import types
    try:
        from antenv.axon_hooks import get_axon_ntff_profile_hook  # noqa: F401
        return
    except ImportError:
        pass
    try:
        from trn_agent_boot.trn_boot import _ntff_profile_via_ctypes
        hook = _ntff_profile_via_ctypes("/opt/axon/libaxon_pjrt.so")
    except Exception:
        hook = None
    mod = types.ModuleType("antenv.axon_hooks")
    state = {"hook": hook}
    mod.get_axon_ntff_profile_hook = lambda: state["hook"]
    mod.set_axon_ntff_profile_hook = lambda h: state.update(hook=h)
    sys.modules["antenv.axon_hooks"] = mod
    import antenv
    antenv.axon_hooks = mod


def kernel_profiled(inputs, trace_cores=None):
    """Like kernel() but with trace=True; returns (out, BassKernelResults)."""
    _install_ntff_hook_shim()
    nc = _get_nc()
    in_maps = _shard_inputs(inputs)
    res = run_bass_kernel_spmd(
        nc, in_maps, core_ids=list(range(N_CORES)), trace=True,
        trace_cores=trace_cores if trace_cores is not None else [0])
    out = np.stack([res.results[i]["out"] for i in range(N_CORES)], axis=0)
    return out.astype(np.float32), res


if __name__ == "__main__":
    import sys
    if "--sim" in sys.argv:
        # quick single-core CoreSim check against the collapsed math
        from concourse.bass_interp import CoreSim
        rng = np.random.default_rng(0)
        x = rng.standard_normal((L, D), dtype=np.float32)
        wv = rng.standard_normal((D, HD), dtype=np.float32) * 0.04
        bv = rng.standard_normal(HD, dtype=np.float32) * 0.04
        fc_w = rng.standard_normal((HD, D), dtype=np.float32) * 0.015
        fc_b = rng.standard_normal(D, dtype=np.float32) * 0.015
        g = rng.standard_normal(D, dtype=np.float32) * 0.3 + 1.0
        b = rng.standard_normal(D, dtype=np.float32) * 0.1

        nc = _get_nc()
        sim = CoreSim(nc, trace=False)
        sim.tensor("x")[:] = x
        sim.tensor("wv")[:] = wv
        sim.tensor("bvT")[:] = np.ascontiguousarray(bv.reshape(KO, P).T)
        sim.tensor("fc_w")[:] = fc_w
        sim.tensor("fc_b")[:] = fc_b[None]
        sim.tensor("ln_g")[:] = g[None]
        sim.tensor("ln_b")[:] = b[None]
        sim.simulate()
        got = np.array(sim.tensor("out"))

        xsum = x.sum(0)
        z = (xsum @ wv + L * bv) @ fc_w + fc_b
        y = x + z[None, :]
        mu = y.mean(-1, keepdims=True)
        var = y.var(-1, keepdims=True)
        want = (y - mu) / np.sqrt(var + EPS) * g + b
        err = np.abs(got - want).max() / np.abs(want).max()
        print("sim absmax rel err:", err)
        assert err < 2e-5, err
        print("SIM PASS")


# revision 10
# speedup vs baseline: 1.1081x; 1.1081x over previous
"""Trainium2 Bass kernel for nn_MultiHeadAttention_26482768347194.

Key algebraic fact: the reference applies softmax over a size-1 trailing
axis, so the attention score matrix is exactly all-ones.  The whole module
collapses (exactly, in real arithmetic) to

    xsum[b]   = sum_l x[b, l, :]                        # (D,)
    t[b]      = xsum[b] @ wv + L * bv                   # (H*D,)
    z[b]      = t[b] @ fc_w + fc_b                      # (D,)
    y[b,l,:]  = x[b,l,:] + z[b]
    out       = LayerNorm(y) * ln_g + ln_b              # over last dim

q/k/tanh/score inputs are mathematically dead.

Sharding (V2): data-parallel over batch for x/out (one batch element per
core) plus tensor-parallel over the h*d axis for the weights (each core
holds a 512-wide slice of wv / 512-tall slice of fc_w).  Core i:
  1. xsum_i from its x shard (TensorE ones-matmul),
  2. AllGather xsum -> (8, 512) on every core,
  3. t-slice and z-partials for ALL batches with its weight slice,
  4. ReduceScatter(add) of the (8, 512) z-partials -> this core's z row,
  5. residual + LayerNorm over its batch, write out.

PE matmuls run in bf16 (fp32 is 2-4x slower on the PE and doubles DMA);
the residual/LN path stays fp32.  Measured end-to-end absmax relative
error vs the fp32 reference: ~2e-3.

This file is self-contained: shapes are hardcoded, no sibling imports.
"""

from contextlib import ExitStack

import numpy as np
import ml_dtypes

import concourse.bass as bass
import concourse.bacc as bacc
import concourse.mybir as mybir
import concourse.tile as tile
from concourse.bass_utils import run_bass_kernel_spmd
from concourse.masks import make_identity

B, L, D, H = 8, 1024, 512, 8
HD = H * D          # 4096
P = 128             # partitions
NT = L // P         # 8 token tiles per core
KD = D // P         # 4 contraction chunks over d
S = D // P          # 4 chunks over the 512-wide hd slice
EPS = 1e-5
N_CORES = 8
RG = [list(range(N_CORES))]

F32 = mybir.dt.float32
BF16 = mybir.dt.bfloat16
AF = mybir.ActivationFunctionType
ALU = mybir.AluOpType


def build_kernel():
    nc = bacc.Bacc("TRN2", target_bir_lowering=False, debug=False,
                   num_devices=N_CORES)

    x_d = nc.dram_tensor("x", [L, D], BF16, kind="ExternalInput")
    wvs_d = nc.dram_tensor("wvs", [D, D], BF16, kind="ExternalInput")
    bvsT_d = nc.dram_tensor("bvsT", [P, S], F32, kind="ExternalInput")
    fcs_d = nc.dram_tensor("fcs", [D, D], BF16, kind="ExternalInput")
    fcb_d = nc.dram_tensor("fc_b", [1, D], F32, kind="ExternalInput")
    g_d = nc.dram_tensor("ln_g", [1, D], F32, kind="ExternalInput")
    b_d = nc.dram_tensor("ln_b", [1, D], F32, kind="ExternalInput")
    out_d = nc.dram_tensor("out", [L, D], F32, kind="ExternalOutput")

    x_v = x_d.ap().rearrange("(t p) d -> t p d", p=P)      # [NT, P, D]
    wvs_v = wvs_d.ap().rearrange("(k p) n -> k p n", p=P)  # [KD, P, D]
    fcs_v = fcs_d.ap().rearrange("(o p) d -> o p d", p=P)  # [S, P, D]
    out_v = out_d.ap().rearrange("(t p) d -> t p d", p=P)  # [NT, P, D]

    with tile.TileContext(nc) as tc, ExitStack() as ctx:
        consts = ctx.enter_context(tc.tile_pool(name="consts", bufs=1))
        work = ctx.enter_context(tc.tile_pool(name="work", bufs=3))
        psum = ctx.enter_context(
            tc.tile_pool(name="psum", bufs=1, space=bass.MemorySpace.PSUM))
        psum4 = ctx.enter_context(
            tc.tile_pool(name="psum4", bufs=2, space=bass.MemorySpace.PSUM))
        dram = ctx.enter_context(
            tc.tile_pool(name="dram", bufs=1, space=bass.MemorySpace.DRAM))

        # ---- constants -------------------------------------------------
        ones_col = consts.tile([P, 1], BF16)     # token-sum matmul lhsT
        nc.gpsimd.memset(ones_col[:], 1.0)
        ones2 = consts.tile([2, P], F32)         # z+fc_b broadcast lhsT
        nc.gpsimd.memset(ones2[:], 1.0)
        eps_t = consts.tile([P, 1], F32)
        nc.gpsimd.memset(eps_t[:], EPS)
        ident_bf = consts.tile([P, P], BF16)
        make_identity(nc, ident_bf)

        bvsT_t = consts.tile([P, S], F32)
        nc.sync.dma_start(bvsT_t[:], bvsT_d.ap())
        bvs1024 = consts.tile([P, S], F32)
        nc.scalar.mul(bvs1024[:], bvsT_t[:], float(L))

        fcb_t = consts.tile([1, D], F32)
        nc.sync.dma_start(fcb_t[:], fcb_d.ap())
        g_t = consts.tile([1, D], F32)
        nc.sync.dma_start(g_t[:], g_d.ap())
        b_t = consts.tile([1, D], F32)
        nc.sync.dma_start(b_t[:], b_d.ap())

        # broadcast ln_g / ln_b to all 128 partitions via a K=1 matmul
        g_bc = consts.tile([P, D], F32)
        b_bc = consts.tile([P, D], F32)
        for src, dst in ((g_t, g_bc), (b_t, b_bc)):
            pb = psum.tile([P, D], F32, tag="bcast")
            nc.tensor.matmul(pb[:], ones2[0:1, :], src[:], start=True, stop=True)
            nc.vector.tensor_copy(dst[:], pb[:])

        # ---- weights ---------------------------------------------------
        wvs_t = consts.tile([P, KD, D], BF16)
        for kd in range(KD):
            nc.sync.dma_start(wvs_t[:, kd, :], wvs_v[kd])
        fcs_t = consts.tile([P, S, D], BF16)
        for os in range(S):
            nc.sync.dma_start(fcs_t[:, os, :], fcs_v[os])

        # ---- load x (stays resident in SBUF) ---------------------------
        x_t = consts.tile([P, NT, D], BF16)
        for t in range(NT):
            nc.sync.dma_start(x_t[:, t, :], x_v[t])

        # ---- phase A: xsum row = sum over tokens of x ------------------
        ps_xsum = psum.tile([1, D], F32, tag="xsum")
        for t in range(NT):
            nc.tensor.matmul(ps_xsum[:], ones_col[:], x_t[:, t, :],
                             start=(t == 0), stop=(t == NT - 1))
        xsum_row = consts.tile([1, D], F32)
        nc.vector.tensor_copy(xsum_row[:], ps_xsum[:])

        xsum_io = dram.tile([1, D], F32)
        nc.sync.dma_start(xsum_io[:], xsum_row[:])
        xg_sh = dram.tile([B, D], F32, addr_space="Shared")
        nc.gpsimd.collective_compute(
            "AllGather", ALU.bypass, replica_groups=RG,
            ins=[xsum_io.opt()], outs=[xg_sh.opt()])

        # ---- phase B: transpose gathered xsums to [d, batch] -----------
        xg_t = consts.tile([B, D], F32)
        nc.sync.dma_start(xg_t[:], xg_sh[:])
        xg_bf = consts.tile([B, D], BF16)
        nc.vector.tensor_copy(xg_bf[:], xg_t[:])
        xsT = consts.tile([P, KD, B], BF16)
        for c in range(KD):
            pxt = psum4.tile([P, B], BF16, tag="xt")
            nc.tensor.transpose(pxt[:], xg_bf[:, c * P:(c + 1) * P],
                                ident_bf[0:B, 0:B])
            nc.vector.tensor_copy(xsT[:, c, :], pxt[:])

        # ---- phase C: t-slice = wvs.T @ xsum.T + L*bvs  ----------------
        tT_sb = consts.tile([P, S, B], BF16)
        for os in range(S):
            ps_t = psum4.tile([P, B], F32, tag="t")
            for kd in range(KD):
                nc.tensor.matmul(ps_t[:],
                                 wvs_t[:, kd, os * P:(os + 1) * P],
                                 xsT[:, kd, :],
                                 start=(kd == 0), stop=(kd == KD - 1))
            nc.scalar.activation(tT_sb[:, os, :], ps_t[:], AF.Identity,
                                 bias=bvs1024[:, os:os + 1], scale=1.0)

        # ---- phase D: z-partials for all batches = t-slice @ fcs -------
        ps_zp = psum.tile([B, D], F32, tag="zp")
        for os in range(S):
            nc.tensor.matmul(ps_zp[:], tT_sb[:, os, :], fcs_t[:, os, :],
                             start=(os == 0), stop=(os == S - 1))
        zp_sb = consts.tile([B, D], F32)
        nc.vector.tensor_copy(zp_sb[:], ps_zp[:])

        zp_io = dram.tile([B, D], F32)
        nc.sync.dma_start(zp_io[:], zp_sb[:])
        zrow_sh = dram.tile([1, D], F32)
        nc.gpsimd.collective_compute(
            "ReduceScatter", ALU.add, replica_groups=RG,
            ins=[zp_io.opt()], outs=[zrow_sh.opt()])

        # ---- phase E: zb = broadcast(z + fc_b) to 128 partitions -------
        zrf = consts.tile([2, D], F32)
        nc.sync.dma_start(zrf[0:1, :], zrow_sh[:])
        nc.sync.dma_start(zrf[1:2, :], fcb_d.ap())
        ps_zb = psum.tile([P, D], F32, tag="bcast")
        nc.tensor.matmul(ps_zb[:], ones2[:], zrf[:], start=True, stop=True)
        zb = consts.tile([P, D], F32)
        nc.vector.tensor_copy(zb[:], ps_zb[:])

        # ---- phase F: residual + layernorm per token tile --------------
        inv_d = 1.0 / D
        for t in range(NT):
            y = work.tile([P, D], F32, tag="y")
            sum_y = work.tile([P, 1], F32, tag="sum_y")
            nc.vector.scalar_tensor_tensor(
                y[:], x_t[:, t, :], 1.0, zb[:],
                op0=ALU.mult, op1=ALU.add, accum_out=sum_y[:])

            sq = work.tile([P, D], BF16, tag="sq")
            sumsq = work.tile([P, 1], F32, tag="sumsq")
            nc.scalar.activation(sq[:], y[:], AF.Square, accum_out=sumsq[:])

            mean = work.tile([P, 1], F32, tag="mean")
            nc.scalar.mul(mean[:], sum_y[:], inv_d)
            msq = work.tile([P, 1], F32, tag="msq")
            nc.vector.tensor_mul(msq[:], mean[:], mean[:])
            var = work.tile([P, 1], F32, tag="var")
            nc.vector.scalar_tensor_tensor(
                var[:], sumsq[:], inv_d, msq[:],
                op0=ALU.mult, op1=ALU.subtract)

            std = work.tile([P, 1], F32, tag="std")
            nc.scalar.activation(std[:], var[:], AF.Sqrt, bias=eps_t[:])
            rstd = work.tile([P, 1], F32, tag="rstd")
            nc.vector.reciprocal(rstd[:], std[:])
            nb = work.tile([P, 1], F32, tag="nb")
            nc.vector.scalar_tensor_tensor(
                nb[:], mean[:], -1.0, rstd[:], op0=ALU.mult, op1=ALU.mult)

            xn = work.tile([P, D], F32, tag="xn")
            nc.scalar.activation(xn[:], y[:], AF.Identity,
                                 bias=nb[:], scale=rstd[:])

            o1 = work.tile([P, D], F32, tag="o1")
            nc.vector.tensor_mul(o1[:], xn[:], g_bc[:])
            o2 = work.tile([P, D], F32, tag="o2")
            nc.vector.tensor_add(o2[:], o1[:], b_bc[:])

            nc.sync.dma_start(out_v[t], o2[:])

    nc.compile()
    return nc


_NC_CACHE = None


def _get_nc():
    global _NC_CACHE
    if _NC_CACHE is None:
        _NC_CACHE = build_kernel()
    return _NC_CACHE


def _shard_inputs(inputs):
    bf = ml_dtypes.bfloat16
    x = np.asarray(inputs["input"], dtype=np.float32)
    wv = np.asarray(inputs["wv"], dtype=np.float32)
    bv = np.asarray(inputs["bv"], dtype=np.float32)
    fc_w = np.asarray(inputs["fc_w"], dtype=np.float32)
    fc_b = np.asarray(inputs["fc_b"], dtype=np.float32)
    ln_g = np.asarray(inputs["ln_g"], dtype=np.float32)
    ln_b = np.asarray(inputs["ln_b"], dtype=np.float32)

    fcb = np.ascontiguousarray(fc_b[None, :])
    g = np.ascontiguousarray(ln_g[None, :])
    b = np.ascontiguousarray(ln_b[None, :])

    in_maps = []
    for i in range(N_CORES):
        sl = slice(i * D, (i + 1) * D)
        in_maps.append({
            "x": np.ascontiguousarray(x[i]).astype(bf),
            "wvs": np.ascontiguousarray(wv[:, sl]).astype(bf),
            "bvsT": np.ascontiguousarray(bv[sl].reshape(S, P).T),
            "fcs": np.ascontiguousarray(fc_w[sl, :]).astype(bf),
            "fc_b": fcb,
            "ln_g": g,
            "ln_b": b,
        })
    return in_maps


def kernel(**inputs) -> np.ndarray:
    nc = _get_nc()
    in_maps = _shard_inputs(inputs)
    res = run_bass_kernel_spmd(nc, in_maps, core_ids=list(range(N_CORES)))
    out = np.stack([res.results[i]["out"] for i in range(N_CORES)], axis=0)
    return out.astype(np.float32)


def _install_ntff_hook_shim():
    """Bridge trn_boot's ctypes NTFF profiler into antenv.axon_hooks,
    which bass_utils imports when trace=True under axon."""
    import sys
    import types
    try:
        from antenv.axon_hooks import get_axon_ntff_profile_hook  # noqa: F401
        return
    except ImportError:
        pass
    try:
        from trn_agent_boot.trn_boot import _ntff_profile_via_ctypes
        hook = _ntff_profile_via_ctypes("/opt/axon/libaxon_pjrt.so")
    except Exception:
        hook = None
    mod = types.ModuleType("antenv.axon_hooks")
    state = {"hook": hook}
    mod.get_axon_ntff_profile_hook = lambda: state["hook"]
    mod.set_axon_ntff_profile_hook = lambda h: state.update(hook=h)
    sys.modules["antenv.axon_hooks"] = mod
    import antenv
    antenv.axon_hooks = mod


def kernel_profiled(inputs, trace_cores=None):
    """Like kernel() but with trace=True; returns (out, BassKernelResults)."""
    _install_ntff_hook_shim()
    nc = _get_nc()
    in_maps = _shard_inputs(inputs)
    res = run_bass_kernel_spmd(
        nc, in_maps, core_ids=list(range(N_CORES)), trace=True,
        trace_cores=trace_cores if trace_cores is not None else [0])
    out = np.stack([res.results[i]["out"] for i in range(N_CORES)], axis=0)
    return out.astype(np.float32), res


if __name__ == "__main__":
    import sys
    if "--sim" in sys.argv:
        # multi-core sim check against the collapsed math
        from concourse.bass_interp import MultiCoreSim
        rng = np.random.default_rng(0)
        x = rng.standard_normal((B, L, D), dtype=np.float32)
        wv = rng.standard_normal((D, HD), dtype=np.float32) * 0.04
        bv = rng.standard_normal(HD, dtype=np.float32) * 0.04
        fc_w = rng.standard_normal((HD, D), dtype=np.float32) * 0.015
        fc_b = rng.standard_normal(D, dtype=np.float32) * 0.015
        g = rng.standard_normal(D, dtype=np.float32) * 0.3 + 1.0
        b = rng.standard_normal(D, dtype=np.float32) * 0.1
        inputs = dict(input=x, wv=wv, bv=bv, fc_w=fc_w, fc_b=fc_b,
                      ln_g=g, ln_b=b)

        nc = _get_nc()
        in_maps = _shard_inputs(inputs)
        sim = MultiCoreSim(nc, N_CORES)
        for i in range(N_CORES):
            for k, v in in_maps[i].items():
                sim.cores[i].tensor(k)[:] = v
        sim.simulate()
        got = np.stack(
            [np.array(sim.cores[i].mem_tensor("out")) for i in range(N_CORES)])

        xsum = x.sum(1)
        z = (xsum @ wv + L * bv) @ fc_w + fc_b
        y = x + z[:, None, :]
        mu = y.mean(-1, keepdims=True)
        var = y.var(-1, keepdims=True)
        want = (y - mu) / np.sqrt(var + EPS) * g + b
        err = np.abs(got - want).max() / np.abs(want).max()
        print("sim absmax rel err:", err)
        assert err < 2e-2, err
        print("SIM PASS")


# revision 11
# speedup vs baseline: 1.1998x; 1.0827x over previous
"""Trainium2 Bass kernel for nn_MultiHeadAttention_26482768347194.

Key algebraic fact: the reference applies softmax over a size-1 trailing
axis, so the attention score matrix is exactly all-ones.  The whole module
collapses (exactly, in real arithmetic) to

    xsum[b]   = sum_l x[b, l, :]                        # (D,)
    t[b]      = xsum[b] @ wv + L * bv                   # (H*D,)
    z[b]      = t[b] @ fc_w + fc_b                      # (D,)
    y[b,l,:]  = x[b,l,:] + z[b]
    out       = LayerNorm(y) * ln_g + ln_b              # over last dim

q/k/tanh/score inputs are mathematically dead.

Sharding (V3): pure data-parallel over batch, one batch element per core,
weights replicated.  Cross-core collectives measured ~70us under this
runtime (launch-skew barrier), so each core runs fully independently:
  1. xsum.T from its x shard (x-stationary PE matmuls against ones),
  2. t.T = wv.T @ xsum.T per 128-wide hd column (wv chunks stationary),
  3. z = t @ fc_w (t.T columns stationary, fc rows moving), + fc_b,
  4. broadcast z to 128 partitions via a K=2 ones-matmul (adds fc_b),
  5. residual + LayerNorm with batched per-tile statistics.

PE matmuls run in bf16 (fp32 is 2-4x slower on the PE and doubles DMA);
the residual/LN statistics stay fp32.  End-to-end absmax relative error
vs the fp32 reference: ~2e-3 (bf16 weight rounding, well inside the
scale-relative gate).

This file is self-contained: shapes are hardcoded, no sibling imports.
"""

from contextlib import ExitStack

import numpy as np
import ml_dtypes

import concourse.bass as bass
import concourse.bacc as bacc
import concourse.mybir as mybir
import concourse.tile as tile
from concourse.bass_utils import run_bass_kernel_spmd

B, L, D, H = 8, 1024, 512, 8
HD = H * D          # 4096
P = 128             # partitions
NT = L // P         # 8 token tiles per core
KD = D // P         # 4 contraction chunks over d
KO = HD // P        # 32 contraction chunks over h*d
EPS = 1e-5
N_CORES = 8

F32 = mybir.dt.float32
BF16 = mybir.dt.bfloat16
AF = mybir.ActivationFunctionType
ALU = mybir.AluOpType

# which engine runs the final "+ ln_b" pass (see LN phase)
LN_B_ON_GPSIMD = True


def build_kernel():
    nc = bacc.Bacc("TRN2", target_bir_lowering=False, debug=False,
                   num_devices=N_CORES)

    x_d = nc.dram_tensor("x", [L, D], BF16, kind="ExternalInput")
    wv_d = nc.dram_tensor("wv", [D, HD], BF16, kind="ExternalInput")
    bvT_d = nc.dram_tensor("bvT", [P, KO], F32, kind="ExternalInput")
    fc_d = nc.dram_tensor("fc_w", [HD, D], BF16, kind="ExternalInput")
    fcb_d = nc.dram_tensor("fc_b", [1, D], F32, kind="ExternalInput")
    g_d = nc.dram_tensor("ln_g", [1, D], F32, kind="ExternalInput")
    b_d = nc.dram_tensor("ln_b", [1, D], F32, kind="ExternalInput")
    out_d = nc.dram_tensor("out", [L, D], F32, kind="ExternalOutput")

    x_v = x_d.ap().rearrange("(t p) d -> t p d", p=P)      # [NT, P, D]
    wv_v = wv_d.ap().rearrange("(k p) n -> k p n", p=P)    # [KD, P, HD]
    fc_v = fc_d.ap().rearrange("(o p) d -> o p d", p=P)    # [KO, P, D]
    out_v = out_d.ap().rearrange("(t p) d -> t p d", p=P)  # [NT, P, D]

    with tile.TileContext(nc) as tc, ExitStack() as ctx:
        consts = ctx.enter_context(tc.tile_pool(name="consts", bufs=1))
        fcp = ctx.enter_context(tc.tile_pool(name="fcp", bufs=1))
        work = ctx.enter_context(tc.tile_pool(name="work", bufs=3))
        ypool = ctx.enter_context(tc.tile_pool(name="ypool", bufs=8))
        psum = ctx.enter_context(
            tc.tile_pool(name="psum", bufs=1, space=bass.MemorySpace.PSUM))

        # ---- constants -------------------------------------------------
        ones_col = consts.tile([P, 1], BF16)     # token-sum matmul rhs
        nc.gpsimd.memset(ones_col[:], 1.0)
        ones2 = consts.tile([2, P], F32)         # z+fc_b broadcast lhsT
        nc.gpsimd.memset(ones2[:], 1.0)
        eps_t = consts.tile([P, 1], F32)
        nc.gpsimd.memset(eps_t[:], EPS)

        bvT_t = consts.tile([P, KO], F32)
        nc.sync.dma_start(bvT_t[:], bvT_d.ap())
        bv1024 = consts.tile([P, KO], F32)
        nc.scalar.mul(bv1024[:], bvT_t[:], float(L))

        fcb_t = consts.tile([1, D], F32)
        nc.sync.dma_start(fcb_t[:], fcb_d.ap())
        g_t = consts.tile([1, D], F32)
        nc.sync.dma_start(g_t[:], g_d.ap())
        b_t = consts.tile([1, D], F32)
        nc.sync.dma_start(b_t[:], b_d.ap())

        # broadcast ln_g / ln_b to all 128 partitions via a K=1 matmul
        g_bc = consts.tile([P, D], F32)
        b_bc = consts.tile([P, D], F32)
        for src, dst in ((g_t, g_bc), (b_t, b_bc)):
            pb = psum.tile([P, D], F32, tag="bcast")
            nc.tensor.matmul(pb[:], ones2[0:1, :], src[:], start=True, stop=True)
            nc.vector.tensor_copy(dst[:], pb[:])

        # ---- weights (bf16, streamed; wv fully resident for col order) -
        wv_t = consts.tile([P, KD, HD], BF16)
        for kd in range(KD):
            for j in range(2):
                sl = slice(j * (HD // 2), (j + 1) * (HD // 2))
                nc.sync.dma_start(wv_t[:, kd, sl], wv_v[kd][:, sl])
        fc_tiles = []
        for gi in range(KO // 4):
            ft = fcp.tile([P, 4, D], BF16, tag=f"fc{gi}")
            for q in range(4):
                nc.sync.dma_start(ft[:, q, :], fc_v[gi * 4 + q])
            fc_tiles.append(ft)

        # ---- load x (stays resident in SBUF) ---------------------------
        x_t = consts.tile([P, NT, D], BF16)
        for t in range(NT):
            nc.sync.dma_start(x_t[:, t, :], x_v[t])

        # ---- phase A: xsumT[d] = sum over tokens of x ------------------
        # lhsT = x tile slice [128 tok, 128 d], rhs = ones -> psum [128 d, 1]
        ps_xs = psum.tile([P, KD], F32, tag="xs")
        for c in range(KD):
            for t in range(NT):
                nc.tensor.matmul(
                    ps_xs[:, c:c + 1],
                    x_t[:, t, c * P:(c + 1) * P],
                    ones_col[:],
                    start=(t == 0), stop=(t == NT - 1))
        xsT = consts.tile([P, KD], BF16)
        nc.vector.tensor_copy(xsT[:], ps_xs[:])

        # ---- phase B+C interleaved: tT columns, then z accumulation ----
        # tT column o: psum[:, o] = sum_kd wv[kd][:, o*128:(o+1)*128].T @ xsT[:, kd]
        # then ACT copies it to SBUF (adding L*bv) so matmul2 can consume it.
        ps_tT = psum.tile([P, KO], F32, tag="tT")
        tT_sb = consts.tile([P, KO], BF16)
        ps_z = psum.tile([1, D], F32, tag="z")
        for o in range(KO):
            for kd in range(KD):
                nc.tensor.matmul(
                    ps_tT[:, o:o + 1],
                    wv_t[:, kd, o * P:(o + 1) * P],
                    xsT[:, kd:kd + 1],
                    start=(kd == 0), stop=(kd == KD - 1))
            nc.scalar.activation(tT_sb[:, o:o + 1], ps_tT[:, o:o + 1],
                                 AF.Identity, bias=bv1024[:, o:o + 1],
                                 scale=1.0)
            nc.tensor.matmul(
                ps_z[:],
                tT_sb[:, o:o + 1],
                fc_tiles[o // 4][:, o % 4, :],
                start=(o == 0), stop=(o == KO - 1),
                skip_group_check=True)

        # ---- phase D: zb = broadcast(z + fc_b) to 128 partitions -------
        zrf = consts.tile([2, D], F32)
        nc.vector.tensor_copy(zrf[0:1, :], ps_z[:])
        nc.sync.dma_start(zrf[1:2, :], fcb_d.ap())
        ps_zb = psum.tile([P, D], F32, tag="bcast")
        nc.tensor.matmul(ps_zb[:], ones2[:], zrf[:], start=True, stop=True)
        zb = consts.tile([P, D], F32)
        nc.vector.tensor_copy(zb[:], ps_zb[:])

        # ---- phase E: residual + layernorm, batched statistics ---------
        inv_d = 1.0 / D
        sums8 = consts.tile([P, NT], F32)
        sumsq8 = consts.tile([P, NT], F32)
        y_tiles = []
        for t in range(NT):
            y = ypool.tile([P, D], F32, tag="y")
            nc.vector.scalar_tensor_tensor(
                y[:], x_t[:, t, :], 1.0, zb[:],
                op0=ALU.mult, op1=ALU.add, accum_out=sums8[:, t:t + 1])
            sq = work.tile([P, D], BF16, tag="sq")
            nc.scalar.activation(sq[:], y[:], AF.Square,
                                 accum_out=sumsq8[:, t:t + 1])
            y_tiles.append(y)

        mean8 = consts.tile([P, NT], F32)
        nc.scalar.mul(mean8[:], sums8[:], inv_d)
        msq8 = consts.tile([P, NT], F32)
        nc.vector.tensor_mul(msq8[:], mean8[:], mean8[:])
        var8 = consts.tile([P, NT], F32)
        nc.vector.scalar_tensor_tensor(
            var8[:], sumsq8[:], inv_d, msq8[:],
            op0=ALU.mult, op1=ALU.subtract)
        std8 = consts.tile([P, NT], F32)
        nc.scalar.activation(std8[:], var8[:], AF.Sqrt, bias=eps_t[:])
        rstd8 = consts.tile([P, NT], F32)
        nc.vector.reciprocal(rstd8[:], std8[:])
        nb8 = consts.tile([P, NT], F32)
        nc.vector.scalar_tensor_tensor(
            nb8[:], mean8[:], -1.0, rstd8[:], op0=ALU.mult, op1=ALU.mult)

        for t in range(NT):
            xn = work.tile([P, D], F32, tag="xn")
            nc.scalar.activation(xn[:], y_tiles[t][:], AF.Identity,
                                 bias=nb8[:, t:t + 1], scale=rstd8[:, t:t + 1])
            o1 = work.tile([P, D], F32, tag="o1")
            nc.vector.tensor_mul(o1[:], xn[:], g_bc[:])
            o2 = work.tile([P, D], F32, tag="o2")
            if LN_B_ON_GPSIMD:
                nc.gpsimd.tensor_add(o2[:], o1[:], b_bc[:])
            else:
                nc.vector.tensor_add(o2[:], o1[:], b_bc[:])
            nc.sync.dma_start(out_v[t], o2[:])

    nc.compile()
    return nc


_NC_CACHE = None


def _get_nc():
    global _NC_CACHE
    if _NC_CACHE is None:
        _NC_CACHE = build_kernel()
    return _NC_CACHE


def _shard_inputs(inputs):
    bf = ml_dtypes.bfloat16
    x = np.asarray(inputs["input"], dtype=np.float32)
    wv = np.ascontiguousarray(np.asarray(inputs["wv"], dtype=np.float32)).astype(bf)
    bv = np.asarray(inputs["bv"], dtype=np.float32)
    fc_w = np.ascontiguousarray(np.asarray(inputs["fc_w"], dtype=np.float32)).astype(bf)
    fc_b = np.asarray(inputs["fc_b"], dtype=np.float32)
    ln_g = np.asarray(inputs["ln_g"], dtype=np.float32)
    ln_b = np.asarray(inputs["ln_b"], dtype=np.float32)

    bvT = np.ascontiguousarray(bv.reshape(KO, P).T)          # [128, 32]
    fcb = np.ascontiguousarray(fc_b[None, :])
    g = np.ascontiguousarray(ln_g[None, :])
    b = np.ascontiguousarray(ln_b[None, :])

    in_maps = []
    for i in range(N_CORES):
        in_maps.append({
            "x": np.ascontiguousarray(x[i]).astype(bf),
            "wv": wv,
            "bvT": bvT,
            "fc_w": fc_w,
            "fc_b": fcb,
            "ln_g": g,
            "ln_b": b,
        })
    return in_maps


def kernel(**inputs) -> np.ndarray:
    nc = _get_nc()
    in_maps = _shard_inputs(inputs)
    res = run_bass_kernel_spmd(nc, in_maps, core_ids=list(range(N_CORES)))
    out = np.stack([res.results[i]["out"] for i in range(N_CORES)], axis=0)
    return out.astype(np.float32)


def _install_ntff_hook_shim():
    """Bridge trn_boot's ctypes NTFF profiler into antenv.axon_hooks,
    which bass_utils imports when trace=True under axon."""
    import sys
    import types
    try:
        from antenv.axon_hooks import get_axon_ntff_profile_hook  # noqa: F401
        return
    except ImportError:
        pass
    try:
        from trn_agent_boot.trn_boot import _ntff_profile_via_ctypes
        hook = _ntff_profile_via_ctypes("/opt/axon/libaxon_pjrt.so")
    except Exception:
        hook = None
    mod = types.ModuleType("antenv.axon_hooks")
    state = {"hook": hook}
    mod.get_axon_ntff_profile_hook = lambda: state["hook"]
    mod.set_axon_ntff_profile_hook = lambda h: state.update(hook=h)
    sys.modules["antenv.axon_hooks"] = mod
    import antenv
    antenv.axon_hooks = mod


def kernel_profiled(inputs, trace_cores=None):
    """Like kernel() but with trace=True; returns (out, BassKernelResults)."""
    _install_ntff_hook_shim()
    nc = _get_nc()
    in_maps = _shard_inputs(inputs)
    res = run_bass_kernel_spmd(
        nc, in_maps, core_ids=list(range(N_CORES)), trace=True,
        trace_cores=trace_cores if trace_cores is not None else [0])
    out = np.stack([res.results[i]["out"] for i in range(N_CORES)], axis=0)
    return out.astype(np.float32), res


if __name__ == "__main__":
    import sys
    if "--sim" in sys.argv:
        # quick single-core CoreSim check against the collapsed math
        from concourse.bass_interp import CoreSim
        rng = np.random.default_rng(0)
        x = rng.standard_normal((B, L, D), dtype=np.float32)
        wv = rng.standard_normal((D, HD), dtype=np.float32) * 0.04
        bv = rng.standard_normal(HD, dtype=np.float32) * 0.04
        fc_w = rng.standard_normal((HD, D), dtype=np.float32) * 0.015
        fc_b = rng.standard_normal(D, dtype=np.float32) * 0.015
        g = rng.standard_normal(D, dtype=np.float32) * 0.3 + 1.0
        b = rng.standard_normal(D, dtype=np.float32) * 0.1
        inputs = dict(input=x, wv=wv, bv=bv, fc_w=fc_w, fc_b=fc_b,
                      ln_g=g, ln_b=b)

        nc = _get_nc()
        in_maps = _shard_inputs(inputs)
        sim = CoreSim(nc, trace=False)
        for k, v in in_maps[0].items():
            sim.tensor(k)[:] = v
        sim.simulate()
        got = np.array(sim.tensor("out"))

        xsum = x[0].sum(0)
        z = (xsum @ wv + L * bv) @ fc_w + fc_b
        y = x[0] + z[None, :]
        mu = y.mean(-1, keepdims=True)
        var = y.var(-1, keepdims=True)
        want = (y - mu) / np.sqrt(var + EPS) * g + b
        err = np.abs(got - want).max() / np.abs(want).max()
        print("sim absmax rel err:", err)
        assert err < 2e-2, err
        print("SIM PASS")


# revision 15
# speedup vs baseline: 1.8043x; 1.5038x over previous
"""Trainium2 Bass kernel for nn_MultiHeadAttention_26482768347194.

Key algebraic fact: the reference applies softmax over a size-1 trailing
axis, so the attention score matrix is exactly all-ones.  The whole module
collapses (exactly, in real arithmetic) to

    xsum[b]   = sum_l x[b, l, :]                        # (D,)
    t[b]      = xsum[b] @ wv + L * bv                   # (H*D,)
    z[b]      = t[b] @ fc_w + fc_b                      # (D,)
    y[b,l,:]  = x[b,l,:] + z[b]
    out       = LayerNorm(y) * ln_g + ln_b              # over last dim

q/k/tanh/score inputs are mathematically dead.

Sharding (V3): pure data-parallel over batch, one batch element per core,
weights replicated.  Cross-core collectives measured ~70us under this
runtime (launch-skew barrier), so each core runs fully independently:
  1. xsum.T from its x shard (x-stationary PE matmuls against ones),
  2. t.T = wv.T @ xsum.T per 128-wide hd column (wv chunks stationary),
  3. z = t @ fc_w (t.T columns stationary, fc rows moving), + fc_b,
  4. broadcast z to 128 partitions via a K=2 ones-matmul (adds fc_b),
  5. residual + LayerNorm with batched per-tile statistics.

PE matmuls run in bf16 (fp32 is 2-4x slower on the PE and doubles DMA);
the residual/LN statistics stay fp32.  End-to-end absmax relative error
vs the fp32 reference: ~2e-3 (bf16 weight rounding, well inside the
scale-relative gate).

This file is self-contained: shapes are hardcoded, no sibling imports.
"""

from contextlib import ExitStack

import numpy as np
import ml_dtypes

import concourse.bass as bass
import concourse.bacc as bacc
import concourse.mybir as mybir
import concourse.tile as tile
from concourse.bass_utils import run_bass_kernel_spmd

B, L, D, H = 8, 1024, 512, 8
HD = H * D          # 4096
P = 128             # partitions
NT = L // P         # 8 token tiles per core
KD = D // P         # 4 contraction chunks over d
KO = HD // P        # 32 contraction chunks over h*d
EPS = 1e-5
N_CORES = 8

F32 = mybir.dt.float32
BF16 = mybir.dt.bfloat16
AF = mybir.ActivationFunctionType
ALU = mybir.AluOpType

# which engine runs the final "+ ln_b" pass (see LN phase).  GpSimd shares
# an SBUF port with the DVE (exclusive lock), so offloading there slows
# BOTH engines to ~1.7us per pass — keep it on the DVE.
LN_B_ON_GPSIMD = False


def build_kernel():
    nc = bacc.Bacc("TRN2", target_bir_lowering=False, debug=False,
                   num_devices=N_CORES)

    x_d = nc.dram_tensor("x", [L, D], BF16, kind="ExternalInput")
    wv_d = nc.dram_tensor("wv", [D, HD], BF16, kind="ExternalInput")
    bvT_d = nc.dram_tensor("bvT", [P, KO], F32, kind="ExternalInput")
    fc_d = nc.dram_tensor("fc_w", [HD, D], BF16, kind="ExternalInput")
    fcb_d = nc.dram_tensor("fc_b", [1, D], F32, kind="ExternalInput")
    g_d = nc.dram_tensor("ln_g", [1, D], F32, kind="ExternalInput")
    b_d = nc.dram_tensor("ln_b", [1, D], F32, kind="ExternalInput")
    out_d = nc.dram_tensor("out", [L, D], F32, kind="ExternalOutput")

    # column-block views: block j carries the data for tT columns 4j..4j+3
    x_v = x_d.ap().rearrange("(u t p) d -> u p t d", u=2, p=P)   # [2, P, 4, D]
    wv_v = wv_d.ap().rearrange("(k p) (j c) -> j p k c", p=P, c=D)  # [8,P,KD,D]
    fc_v = fc_d.ap().rearrange("(j q p) d -> j p q d", q=4, p=P)    # [8,P,4,D]
    out_v = out_d.ap().rearrange("(t p) d -> t p d", p=P)        # [NT, P, D]

    with tile.TileContext(nc) as tc, ExitStack() as ctx:
        consts = ctx.enter_context(tc.tile_pool(name="consts", bufs=1))
        fcp = ctx.enter_context(tc.tile_pool(name="fcp", bufs=1))
        work = ctx.enter_context(tc.tile_pool(name="work", bufs=3))
        ypool = ctx.enter_context(tc.tile_pool(name="ypool", bufs=8))
        psum = ctx.enter_context(
            tc.tile_pool(name="psum", bufs=1, space=bass.MemorySpace.PSUM))

        # ---- constants -------------------------------------------------
        ones_col = consts.tile([P, 1], BF16)     # token-sum matmul rhs
        nc.gpsimd.memset(ones_col[:], 1.0)
        ones2 = consts.tile([2, P], F32)         # z+fc_b broadcast lhsT
        nc.gpsimd.memset(ones2[:], 1.0)
        eps_t = consts.tile([P, 1], F32)
        nc.gpsimd.memset(eps_t[:], EPS)

        bvT_t = consts.tile([P, KO], F32)
        nc.sync.dma_start(bvT_t[:], bvT_d.ap())
        bv1024 = consts.tile([P, KO], F32)
        nc.scalar.mul(bv1024[:], bvT_t[:], float(L))

        fcb_t = consts.tile([1, D], F32)
        nc.sync.dma_start(fcb_t[:], fcb_d.ap())
        g_t = consts.tile([1, D], F32)
        nc.sync.dma_start(g_t[:], g_d.ap())
        b_t = consts.tile([1, D], F32)
        nc.sync.dma_start(b_t[:], b_d.ap())

        # broadcast ln_g / ln_b to all 128 partitions via a K=1 matmul
        g_bc = consts.tile([P, D], F32)
        b_bc = consts.tile([P, D], F32)
        for src, dst in ((g_t, g_bc), (b_t, b_bc)):
            pb = psum.tile([P, D], F32, tag="bcast")
            nc.tensor.matmul(pb[:], ones2[0:1, :], src[:], start=True, stop=True)
            nc.vector.tensor_copy(dst[:], pb[:])

        # ---- load x (stays resident in SBUF) ---------------------------
        x_t = consts.tile([P, NT, D], BF16)
        for u in range(2):
            nc.gpsimd.dma_start(x_t[:, u * 4:(u + 1) * 4, :], x_v[u])

        # ---- phase A: xsumT[d] = sum over tokens of x ------------------
        # lhsT = x tile slice [128 tok, 128 d], rhs = ones -> psum [128 d, 1]
        ps_xs = psum.tile([P, KD], F32, tag="xs")
        for c in range(KD):
            for t in range(NT):
                nc.tensor.matmul(
                    ps_xs[:, c:c + 1],
                    x_t[:, t, c * P:(c + 1) * P],
                    ones_col[:],
                    start=(t == 0), stop=(t == NT - 1))
        xsT = consts.tile([P, KD], BF16)
        nc.vector.tensor_copy(xsT[:], ps_xs[:])

        # ---- phase B+C interleaved, streamed by 512-wide hd blocks -----
        # block j: DMA wv cols [4j*128,(4j+4)*128) + fc rows likewise, then
        #   tT col o = sum_kd wv[:, kd, oc*128:..].T @ xsT[:, kd]   (psum)
        #   ACT copies col to SBUF adding L*bv, matmul2 accumulates z.
        ps_tT = psum.tile([P, KO], F32, tag="tT")
        tT_sb = consts.tile([P, KO], BF16)
        ps_z = psum.tile([1, D], F32, tag="z")
        for j in range(KO // 4):
            wv_bt = work.tile([P, KD, D], BF16, tag="wvb")
            nc.sync.dma_start(wv_bt[:], wv_v[j])
            fc_bt = work.tile([P, 4, D], BF16, tag="fcb")
            nc.sync.dma_start(fc_bt[:], fc_v[j])
            for oc in range(4):
                o = 4 * j + oc
                for kd in range(KD):
                    nc.tensor.matmul(
                        ps_tT[:, o:o + 1],
                        wv_bt[:, kd, oc * P:(oc + 1) * P],
                        xsT[:, kd:kd + 1],
                        start=(kd == 0), stop=(kd == KD - 1))
                nc.scalar.activation(tT_sb[:, o:o + 1], ps_tT[:, o:o + 1],
                                     AF.Identity, bias=bv1024[:, o:o + 1],
                                     scale=1.0)
                nc.tensor.matmul(
                    ps_z[:],
                    tT_sb[:, o:o + 1],
                    fc_bt[:, oc, :],
                    start=(o == 0), stop=(o == KO - 1),
                    skip_group_check=True)

        # ---- phase D: zb = broadcast(z + fc_b) to 128 partitions -------
        zrf = consts.tile([2, D], F32)
        nc.vector.tensor_copy(zrf[0:1, :], ps_z[:])
        nc.sync.dma_start(zrf[1:2, :], fcb_d.ap())
        ps_zb = psum.tile([P, D], F32, tag="bcast")
        nc.tensor.matmul(ps_zb[:], ones2[:], zrf[:], start=True, stop=True)
        zb = consts.tile([P, D], F32)
        nc.vector.tensor_copy(zb[:], ps_zb[:])

        # ---- phase E: residual + layernorm, batched statistics ---------
        inv_d = 1.0 / D
        sums8 = consts.tile([P, NT], F32)
        sumsq8 = consts.tile([P, NT], F32)
        y_tiles = []
        for t in range(NT):
            y = ypool.tile([P, D], F32, tag="y")
            nc.vector.scalar_tensor_tensor(
                y[:], x_t[:, t, :], 1.0, zb[:],
                op0=ALU.mult, op1=ALU.add, accum_out=sums8[:, t:t + 1])
            sq = work.tile([P, D], BF16, tag="sq")
            nc.scalar.activation(sq[:], y[:], AF.Square,
                                 accum_out=sumsq8[:, t:t + 1])
            y_tiles.append(y)

        mean8 = consts.tile([P, NT], F32)
        nc.scalar.mul(mean8[:], sums8[:], inv_d)
        msq8 = consts.tile([P, NT], F32)
        nc.vector.tensor_mul(msq8[:], mean8[:], mean8[:])
        var8 = consts.tile([P, NT], F32)
        nc.vector.scalar_tensor_tensor(
            var8[:], sumsq8[:], inv_d, msq8[:],
            op0=ALU.mult, op1=ALU.subtract)
        std8 = consts.tile([P, NT], F32)
        nc.scalar.activation(std8[:], var8[:], AF.Sqrt, bias=eps_t[:])
        rstd8 = consts.tile([P, NT], F32)
        nc.vector.reciprocal(rstd8[:], std8[:])
        nb8 = consts.tile([P, NT], F32)
        nc.vector.scalar_tensor_tensor(
            nb8[:], mean8[:], -1.0, rstd8[:], op0=ALU.mult, op1=ALU.mult)

        for t in range(NT):
            xn = work.tile([P, D], F32, tag="xn")
            nc.scalar.activation(xn[:], y_tiles[t][:], AF.Identity,
                                 bias=nb8[:, t:t + 1], scale=rstd8[:, t:t + 1])
            o1 = work.tile([P, D], F32, tag="o1")
            nc.vector.tensor_mul(o1[:], xn[:], g_bc[:])
            o2 = work.tile([P, D], F32, tag="o2")
            if LN_B_ON_GPSIMD:
                nc.gpsimd.tensor_add(o2[:], o1[:], b_bc[:])
            else:
                nc.vector.tensor_add(o2[:], o1[:], b_bc[:])
            nc.sync.dma_start(out_v[t], o2[:])

    nc.compile()
    return nc


_NC_CACHE = None


def _get_nc():
    global _NC_CACHE
    if _NC_CACHE is None:
        _NC_CACHE = build_kernel()
    return _NC_CACHE


def _shard_inputs(inputs):
    bf = ml_dtypes.bfloat16
    x = np.asarray(inputs["input"], dtype=np.float32)
    wv = np.ascontiguousarray(np.asarray(inputs["wv"], dtype=np.float32)).astype(bf)
    bv = np.asarray(inputs["bv"], dtype=np.float32)
    fc_w = np.ascontiguousarray(np.asarray(inputs["fc_w"], dtype=np.float32)).astype(bf)
    fc_b = np.asarray(inputs["fc_b"], dtype=np.float32)
    ln_g = np.asarray(inputs["ln_g"], dtype=np.float32)
    ln_b = np.asarray(inputs["ln_b"], dtype=np.float32)

    bvT = np.ascontiguousarray(bv.reshape(KO, P).T)          # [128, 32]
    fcb = np.ascontiguousarray(fc_b[None, :])
    g = np.ascontiguousarray(ln_g[None, :])
    b = np.ascontiguousarray(ln_b[None, :])

    in_maps = []
    for i in range(N_CORES):
        in_maps.append({
            "x": np.ascontiguousarray(x[i]).astype(bf),
            "wv": wv,
            "bvT": bvT,
            "fc_w": fc_w,
            "fc_b": fcb,
            "ln_g": g,
            "ln_b": b,
        })
    return in_maps


def kernel(**inputs) -> np.ndarray:
    nc = _get_nc()
    in_maps = _shard_inputs(inputs)
    res = run_bass_kernel_spmd(nc, in_maps, core_ids=list(range(N_CORES)))
    out = np.stack([res.results[i]["out"] for i in range(N_CORES)], axis=0)
    return out.astype(np.float32)


def _install_ntff_hook_shim():
    """Bridge trn_boot's ctypes NTFF profiler into antenv.axon_hooks,
    which bass_utils imports when trace=True under axon."""
    import sys
    import types
    try:
        from antenv.axon_hooks import get_axon_ntff_profile_hook  # noqa: F401
        return
    except ImportError:
        pass
    try:
        from trn_agent_boot.trn_boot import _ntff_profile_via_ctypes
        hook = _ntff_profile_via_ctypes("/opt/axon/libaxon_pjrt.so")
    except Exception:
        hook = None
    mod = types.ModuleType("antenv.axon_hooks")
    state = {"hook": hook}
    mod.get_axon_ntff_profile_hook = lambda: state["hook"]
    mod.set_axon_ntff_profile_hook = lambda h: state.update(hook=h)
    sys.modules["antenv.axon_hooks"] = mod
    import antenv
    antenv.axon_hooks = mod


def kernel_profiled(inputs, trace_cores=None):
    """Like kernel() but with trace=True; returns (out, BassKernelResults)."""
    _install_ntff_hook_shim()
    nc = _get_nc()
    in_maps = _shard_inputs(inputs)
    res = run_bass_kernel_spmd(
        nc, in_maps, core_ids=list(range(N_CORES)), trace=True,
        trace_cores=trace_cores if trace_cores is not None else [0])
    out = np.stack([res.results[i]["out"] for i in range(N_CORES)], axis=0)
    return out.astype(np.float32), res


if __name__ == "__main__":
    import sys
    if "--sim" in sys.argv:
        # quick single-core CoreSim check against the collapsed math
        from concourse.bass_interp import CoreSim
        rng = np.random.default_rng(0)
        x = rng.standard_normal((B, L, D), dtype=np.float32)
        wv = rng.standard_normal((D, HD), dtype=np.float32) * 0.04
        bv = rng.standard_normal(HD, dtype=np.float32) * 0.04
        fc_w = rng.standard_normal((HD, D), dtype=np.float32) * 0.015
        fc_b = rng.standard_normal(D, dtype=np.float32) * 0.015
        g = rng.standard_normal(D, dtype=np.float32) * 0.3 + 1.0
        b = rng.standard_normal(D, dtype=np.float32) * 0.1
        inputs = dict(input=x, wv=wv, bv=bv, fc_w=fc_w, fc_b=fc_b,
                      ln_g=g, ln_b=b)

        nc = _get_nc()
        in_maps = _shard_inputs(inputs)
        sim = CoreSim(nc, trace=False)
        for k, v in in_maps[0].items():
            sim.tensor(k)[:] = v
        sim.simulate()
        got = np.array(sim.tensor("out"))

        xsum = x[0].sum(0)
        z = (xsum @ wv + L * bv) @ fc_w + fc_b
        y = x[0] + z[None, :]
        mu = y.mean(-1, keepdims=True)
        var = y.var(-1, keepdims=True)
        want = (y - mu) / np.sqrt(var + EPS) * g + b
        err = np.abs(got - want).max() / np.abs(want).max()
        print("sim absmax rel err:", err)
        assert err < 2e-2, err
        print("SIM PASS")


# revision 20
# speedup vs baseline: 1.8120x; 1.0043x over previous
"""Trainium2 Bass kernel for nn_MultiHeadAttention_26482768347194.

Key algebraic fact: the reference applies softmax over a size-1 trailing
axis, so the attention score matrix is exactly all-ones.  The whole module
collapses (exactly, in real arithmetic) to

    xsum[b]   = sum_l x[b, l, :]                        # (D,)
    t[b]      = xsum[b] @ wv + L * bv                   # (H*D,)
    z[b]      = t[b] @ fc_w + fc_b                      # (D,)
    y[b,l,:]  = x[b,l,:] + z[b]
    out       = LayerNorm(y) * ln_g + ln_b              # over last dim

q/k/tanh/score inputs are mathematically dead.

Sharding (V3): pure data-parallel over batch, one batch element per core,
weights replicated.  Cross-core collectives measured ~70us under this
runtime (launch-skew barrier), so each core runs fully independently:
  1. xsum.T from its x shard (x-stationary PE matmuls against ones),
  2. t.T = wv.T @ xsum.T per 128-wide hd column (wv chunks stationary),
  3. z = t @ fc_w (t.T columns stationary, fc rows moving), + fc_b,
  4. broadcast z to 128 partitions via a K=2 ones-matmul (adds fc_b),
  5. residual + LayerNorm with batched per-tile statistics.

PE matmuls run in bf16 (fp32 is 2-4x slower on the PE and doubles DMA);
the residual/LN statistics stay fp32.  End-to-end absmax relative error
vs the fp32 reference: ~2e-3 (bf16 weight rounding, well inside the
scale-relative gate).

This file is self-contained: shapes are hardcoded, no sibling imports.
"""

from contextlib import ExitStack

import numpy as np
import ml_dtypes

import concourse.bass as bass
import concourse.bacc as bacc
import concourse.mybir as mybir
import concourse.tile as tile
from concourse.bass_utils import run_bass_kernel_spmd

B, L, D, H = 8, 1024, 512, 8
HD = H * D          # 4096
P = 128             # partitions
NT = L // P         # 8 token tiles per core
KD = D // P         # 4 contraction chunks over d
KO = HD // P        # 32 contraction chunks over h*d
EPS = 1e-5
N_CORES = 8

F32 = mybir.dt.float32
BF16 = mybir.dt.bfloat16
AF = mybir.ActivationFunctionType
ALU = mybir.AluOpType

# which engine runs the final "+ ln_b" pass (see LN phase).  GpSimd shares
# an SBUF port with the DVE (exclusive lock), so offloading there slows
# BOTH engines to ~1.7us per pass — keep it on the DVE.
LN_B_ON_GPSIMD = False


def build_kernel():
    nc = bacc.Bacc("TRN2", target_bir_lowering=False, debug=False,
                   num_devices=N_CORES)

    # x / wv / fc arrive pre-blocked from the host so that every big DMA
    # reads a fully contiguous region:
    #   x[u, p, t, d]  = x_orig[(u*4 + t)*128 + p, d]          (2 x 1MB)
    #   wv[j, p, k, c] = wv_orig[k*128 + p, j*512 + c]         (8 x 1MB)
    #   fc[j, p, q, d] = fc_orig[(j*4 + q)*128 + p, d]         (8 x 1MB)
    x_d = nc.dram_tensor("x", [2, P, 4, D], BF16, kind="ExternalInput")
    wv_d = nc.dram_tensor("wv", [KO // 4, P, KD, D], BF16, kind="ExternalInput")
    bvT_d = nc.dram_tensor("bvT", [P, KO], F32, kind="ExternalInput")
    fc_d = nc.dram_tensor("fc_w", [KO // 4, P, 4, D], BF16, kind="ExternalInput")
    fcb_d = nc.dram_tensor("fc_b", [1, D], F32, kind="ExternalInput")
    g_d = nc.dram_tensor("ln_g", [1, D], F32, kind="ExternalInput")
    b_d = nc.dram_tensor("ln_b", [1, D], F32, kind="ExternalInput")
    out_d = nc.dram_tensor("out", [L, D], F32, kind="ExternalOutput")

    x_v = x_d.ap()                                               # [2, P, 4, D]
    wv_v = wv_d.ap()                                             # [8, P, KD, D]
    fc_v = fc_d.ap()                                             # [8, P, 4, D]
    out_v = out_d.ap().rearrange("(t p) d -> t p d", p=P)        # [NT, P, D]

    with tile.TileContext(nc) as tc, ExitStack() as ctx:
        consts = ctx.enter_context(tc.tile_pool(name="consts", bufs=1))
        fcp = ctx.enter_context(tc.tile_pool(name="fcp", bufs=1))
        work = ctx.enter_context(tc.tile_pool(name="work", bufs=3))
        ypool = ctx.enter_context(tc.tile_pool(name="ypool", bufs=8))
        psum = ctx.enter_context(
            tc.tile_pool(name="psum", bufs=1, space=bass.MemorySpace.PSUM))

        # ---- constants -------------------------------------------------
        ones_col = consts.tile([P, 1], BF16)     # token-sum matmul rhs
        nc.gpsimd.memset(ones_col[:], 1.0)
        ones2 = consts.tile([2, P], F32)         # z+fc_b broadcast lhsT
        nc.gpsimd.memset(ones2[:], 1.0)
        eps_t = consts.tile([P, 1], F32)
        nc.gpsimd.memset(eps_t[:], EPS)

        bvT_t = consts.tile([P, KO], F32)
        nc.sync.dma_start(bvT_t[:], bvT_d.ap())
        bv1024 = consts.tile([P, KO], F32)
        nc.scalar.mul(bv1024[:], bvT_t[:], float(L))

        fcb_t = consts.tile([1, D], F32)
        nc.sync.dma_start(fcb_t[:], fcb_d.ap())
        g_t = consts.tile([1, D], F32)
        nc.sync.dma_start(g_t[:], g_d.ap())
        b_t = consts.tile([1, D], F32)
        nc.sync.dma_start(b_t[:], b_d.ap())

        # broadcast ln_g / ln_b to all 128 partitions via a K=1 matmul
        g_bc = consts.tile([P, D], F32)
        b_bc = consts.tile([P, D], F32)
        for src, dst in ((g_t, g_bc), (b_t, b_bc)):
            pb = psum.tile([P, D], F32, tag="bcast")
            nc.tensor.matmul(pb[:], ones2[0:1, :], src[:], start=True, stop=True)
            nc.vector.tensor_copy(dst[:], pb[:])

        # ---- load x (stays resident in SBUF) ---------------------------
        x_t = consts.tile([P, NT, D], BF16)
        for u in range(2):
            nc.sync.dma_start(x_t[:, u * 4:(u + 1) * 4, :], x_v[u])

        # ---- phase A: xsumT[d] = sum over tokens of x ------------------
        # lhsT = x tile slice [128 tok, 128 d], rhs = ones -> psum [128 d, 1]
        ps_xs = psum.tile([P, KD], F32, tag="xs")
        for c in range(KD):
            for t in range(NT):
                nc.tensor.matmul(
                    ps_xs[:, c:c + 1],
                    x_t[:, t, c * P:(c + 1) * P],
                    ones_col[:],
                    start=(t == 0), stop=(t == NT - 1))
        xsT = consts.tile([P, KD], BF16)
        nc.vector.tensor_copy(xsT[:], ps_xs[:])

        # ---- phase B+C interleaved, streamed by 512-wide hd blocks -----
        # block j: DMA wv cols [4j*128,(4j+4)*128) + fc rows likewise, then
        #   tT col o = sum_kd wv[:, kd, oc*128:..].T @ xsT[:, kd]   (psum)
        #   ACT copies col to SBUF adding L*bv, matmul2 accumulates z.
        ps_tT = psum.tile([P, KO], F32, tag="tT")
        tT_sb = consts.tile([P, KO], BF16)
        ps_z = psum.tile([1, D], F32, tag="z")
        for j in range(KO // 4):
            wv_bt = work.tile([P, KD, D], BF16, tag="wvb")
            nc.sync.dma_start(wv_bt[:], wv_v[j])
            fc_bt = work.tile([P, 4, D], BF16, tag="fcb")
            nc.sync.dma_start(fc_bt[:], fc_v[j])
            for oc in range(4):
                o = 4 * j + oc
                for kd in range(KD):
                    nc.tensor.matmul(
                        ps_tT[:, o:o + 1],
                        wv_bt[:, kd, oc * P:(oc + 1) * P],
                        xsT[:, kd:kd + 1],
                        start=(kd == 0), stop=(kd == KD - 1))
                nc.scalar.activation(tT_sb[:, o:o + 1], ps_tT[:, o:o + 1],
                                     AF.Identity, bias=bv1024[:, o:o + 1],
                                     scale=1.0)
                nc.tensor.matmul(
                    ps_z[:],
                    tT_sb[:, o:o + 1],
                    fc_bt[:, oc, :],
                    start=(o == 0), stop=(o == KO - 1),
                    skip_group_check=True)

        # ---- phase D: zb = broadcast(z + fc_b) to 128 partitions -------
        zrf = consts.tile([2, D], F32)
        nc.vector.tensor_copy(zrf[0:1, :], ps_z[:])
        nc.sync.dma_start(zrf[1:2, :], fcb_d.ap())
        ps_zb = psum.tile([P, D], F32, tag="bcast")
        nc.tensor.matmul(ps_zb[:], ones2[:], zrf[:], start=True, stop=True)
        zb = consts.tile([P, D], F32)
        nc.vector.tensor_copy(zb[:], ps_zb[:])

        # ---- phase E: residual + layernorm, batched statistics ---------
        inv_d = 1.0 / D
        sums8 = consts.tile([P, NT], F32)
        sumsq8 = consts.tile([P, NT], F32)
        y_tiles = []
        for t in range(NT):
            y = ypool.tile([P, D], F32, tag="y")
            nc.vector.scalar_tensor_tensor(
                y[:], x_t[:, t, :], 1.0, zb[:],
                op0=ALU.mult, op1=ALU.add, accum_out=sums8[:, t:t + 1])
            sq = work.tile([P, D], BF16, tag="sq")
            nc.scalar.activation(sq[:], y[:], AF.Square,
                                 accum_out=sumsq8[:, t:t + 1])
            y_tiles.append(y)

        mean8 = consts.tile([P, NT], F32)
        nc.scalar.mul(mean8[:], sums8[:], inv_d)
        msq8 = consts.tile([P, NT], F32)
        nc.vector.tensor_mul(msq8[:], mean8[:], mean8[:])
        var8 = consts.tile([P, NT], F32)
        nc.vector.scalar_tensor_tensor(
            var8[:], sumsq8[:], inv_d, msq8[:],
            op0=ALU.mult, op1=ALU.subtract)
        std8 = consts.tile([P, NT], F32)
        nc.scalar.activation(std8[:], var8[:], AF.Sqrt, bias=eps_t[:])
        rstd8 = consts.tile([P, NT], F32)
        nc.vector.reciprocal(rstd8[:], std8[:])
        nb8 = consts.tile([P, NT], F32)
        nc.vector.scalar_tensor_tensor(
            nb8[:], mean8[:], -1.0, rstd8[:], op0=ALU.mult, op1=ALU.mult)

        for t in range(NT):
            xn = work.tile([P, D], F32, tag="xn", bufs=4)
            nc.scalar.activation(xn[:], y_tiles[t][:], AF.Identity,
                                 bias=nb8[:, t:t + 1], scale=rstd8[:, t:t + 1])
            o1 = work.tile([P, D], F32, tag="o1")
            nc.vector.tensor_mul(o1[:], xn[:], g_bc[:])
            o2 = work.tile([P, D], F32, tag="o2")
            if LN_B_ON_GPSIMD:
                nc.gpsimd.tensor_add(o2[:], o1[:], b_bc[:])
            else:
                nc.vector.tensor_add(o2[:], o1[:], b_bc[:])
            nc.sync.dma_start(out_v[t], o2[:])

    nc.compile()
    return nc


_NC_CACHE = None


def _get_nc():
    global _NC_CACHE
    if _NC_CACHE is None:
        _NC_CACHE = build_kernel()
    return _NC_CACHE


def _shard_inputs(inputs):
    bf = ml_dtypes.bfloat16
    x = np.asarray(inputs["input"], dtype=np.float32)
    wv = np.asarray(inputs["wv"], dtype=np.float32)
    bv = np.asarray(inputs["bv"], dtype=np.float32)
    fc_w = np.asarray(inputs["fc_w"], dtype=np.float32)
    fc_b = np.asarray(inputs["fc_b"], dtype=np.float32)
    ln_g = np.asarray(inputs["ln_g"], dtype=np.float32)
    ln_b = np.asarray(inputs["ln_b"], dtype=np.float32)

    # blocked layouts (see build_kernel): each 1MB DMA is contiguous
    wv_bl = np.ascontiguousarray(
        wv.reshape(KD, P, KO // 4, D).transpose(2, 1, 0, 3)).astype(bf)
    fc_bl = np.ascontiguousarray(
        fc_w.reshape(KO // 4, 4, P, D).transpose(0, 2, 1, 3)).astype(bf)
    bvT = np.ascontiguousarray(bv.reshape(KO, P).T)          # [128, 32]
    fcb = np.ascontiguousarray(fc_b[None, :])
    g = np.ascontiguousarray(ln_g[None, :])
    b = np.ascontiguousarray(ln_b[None, :])

    in_maps = []
    for i in range(N_CORES):
        x_bl = np.ascontiguousarray(
            x[i].reshape(2, 4, P, D).transpose(0, 2, 1, 3)).astype(bf)
        in_maps.append({
            "x": x_bl,
            "wv": wv_bl,
            "bvT": bvT,
            "fc_w": fc_bl,
            "fc_b": fcb,
            "ln_g": g,
            "ln_b": b,
        })
    return in_maps


def kernel(**inputs) -> np.ndarray:
    nc = _get_nc()
    in_maps = _shard_inputs(inputs)
    res = run_bass_kernel_spmd(nc, in_maps, core_ids=list(range(N_CORES)))
    out = np.stack([res.results[i]["out"] for i in range(N_CORES)], axis=0)
    return out.astype(np.float32)


def _install_ntff_hook_shim():
    """Bridge trn_boot's ctypes NTFF profiler into antenv.axon_hooks,
    which bass_utils imports when trace=True under axon."""
    import sys
    import types
    try:
        from antenv.axon_hooks import get_axon_ntff_profile_hook  # noqa: F401
        return
    except ImportError:
        pass
    try:
        from trn_agent_boot.trn_boot import _ntff_profile_via_ctypes
        hook = _ntff_profile_via_ctypes("/opt/axon/libaxon_pjrt.so")
    except Exception:
        hook = None
    mod = types.ModuleType("antenv.axon_hooks")
    state = {"hook": hook}
    mod.get_axon_ntff_profile_hook = lambda: state["hook"]
    mod.set_axon_ntff_profile_hook = lambda h: state.update(hook=h)
    sys.modules["antenv.axon_hooks"] = mod
    import antenv
    antenv.axon_hooks = mod


def kernel_profiled(inputs, trace_cores=None):
    """Like kernel() but with trace=True; returns (out, BassKernelResults)."""
    _install_ntff_hook_shim()
    nc = _get_nc()
    in_maps = _shard_inputs(inputs)
    res = run_bass_kernel_spmd(
        nc, in_maps, core_ids=list(range(N_CORES)), trace=True,
        trace_cores=trace_cores if trace_cores is not None else [0])
    out = np.stack([res.results[i]["out"] for i in range(N_CORES)], axis=0)
    return out.astype(np.float32), res


if __name__ == "__main__":
    import sys
    if "--sim" in sys.argv:
        # quick single-core CoreSim check against the collapsed math
        from concourse.bass_interp import CoreSim
        rng = np.random.default_rng(0)
        x = rng.standard_normal((B, L, D), dtype=np.float32)
        wv = rng.standard_normal((D, HD), dtype=np.float32) * 0.04
        bv = rng.standard_normal(HD, dtype=np.float32) * 0.04
        fc_w = rng.standard_normal((HD, D), dtype=np.float32) * 0.015
        fc_b = rng.standard_normal(D, dtype=np.float32) * 0.015
        g = rng.standard_normal(D, dtype=np.float32) * 0.3 + 1.0
        b = rng.standard_normal(D, dtype=np.float32) * 0.1
        inputs = dict(input=x, wv=wv, bv=bv, fc_w=fc_w, fc_b=fc_b,
                      ln_g=g, ln_b=b)

        nc = _get_nc()
        in_maps = _shard_inputs(inputs)
        sim = CoreSim(nc, trace=False)
        for k, v in in_maps[0].items():
            sim.tensor(k)[:] = v
        sim.simulate()
        got = np.array(sim.tensor("out"))

        xsum = x[0].sum(0)
        z = (xsum @ wv + L * bv) @ fc_w + fc_b
        y = x[0] + z[None, :]
        mu = y.mean(-1, keepdims=True)
        var = y.var(-1, keepdims=True)
        want = (y - mu) / np.sqrt(var + EPS) * g + b
        err = np.abs(got - want).max() / np.abs(want).max()
        print("sim absmax rel err:", err)
        assert err < 2e-2, err
        print("SIM PASS")


# revision 25
# speedup vs baseline: 1.8758x; 1.0352x over previous
"""Trainium2 Bass kernel for nn_MultiHeadAttention_26482768347194.

Key algebraic fact: the reference applies softmax over a size-1 trailing
axis, so the attention score matrix is exactly all-ones.  The whole module
collapses (exactly, in real arithmetic) to

    xsum[b]   = sum_l x[b, l, :]                        # (D,)
    t[b]      = xsum[b] @ wv + L * bv                   # (H*D,)
    z[b]      = t[b] @ fc_w + fc_b                      # (D,)
    y[b,l,:]  = x[b,l,:] + z[b]
    out       = LayerNorm(y) * ln_g + ln_b              # over last dim

q/k/tanh/score inputs are mathematically dead.

Sharding (V3): pure data-parallel over batch, one batch element per core,
weights replicated.  Cross-core collectives measured ~70us under this
runtime (launch-skew barrier), so each core runs fully independently:
  1. xsum.T from its x shard (x-stationary PE matmuls against ones),
  2. t.T = wv.T @ xsum.T per 128-wide hd column (wv chunks stationary),
  3. z = t @ fc_w (t.T columns stationary, fc rows moving), + fc_b,
  4. broadcast z to 128 partitions via a K=2 ones-matmul (adds fc_b),
  5. residual + LayerNorm with batched per-tile statistics.

PE matmuls run in bf16 (fp32 is 2-4x slower on the PE and doubles DMA);
the residual/LN statistics stay fp32.  End-to-end absmax relative error
vs the fp32 reference: ~2e-3 (bf16 weight rounding, well inside the
scale-relative gate).

This file is self-contained: shapes are hardcoded, no sibling imports.
"""

from contextlib import ExitStack

import numpy as np
import ml_dtypes

import concourse.bass as bass
import concourse.bacc as bacc
import concourse.mybir as mybir
import concourse.tile as tile
from concourse.bass_utils import run_bass_kernel_spmd

B, L, D, H = 8, 1024, 512, 8
HD = H * D          # 4096
P = 128             # partitions
NT = L // P         # 8 token tiles per core
KD = D // P         # 4 contraction chunks over d
KO = HD // P        # 32 contraction chunks over h*d
EPS = 1e-5
N_CORES = 8

F32 = mybir.dt.float32
BF16 = mybir.dt.bfloat16
AF = mybir.ActivationFunctionType
ALU = mybir.AluOpType

# which engine runs the final "+ ln_b" pass (see LN phase).  GpSimd shares
# an SBUF port with the DVE (exclusive lock), so offloading there slows
# BOTH engines to ~1.7us per pass — keep it on the DVE.
LN_B_ON_GPSIMD = False


def build_kernel():
    nc = bacc.Bacc("TRN2", target_bir_lowering=False, debug=False,
                   num_devices=N_CORES)

    # x / wv / fc arrive pre-blocked from the host so that every big DMA
    # reads a fully contiguous region:
    #   x[u, p, t, d]  = x_orig[(u*4 + t)*128 + p, d]          (2 x 1MB)
    #   wv[j, p, k, c] = wv_orig[k*128 + p, j*512 + c]         (8 x 1MB)
    #   fc[j, p, q, d] = fc_orig[(j*4 + q)*128 + p, d]         (8 x 1MB)
    #   xT[c, p, l]    = x_orig[l, c*128 + p]                  (4 x 256KB)
    x_d = nc.dram_tensor("x", [2, P, 4, D], BF16, kind="ExternalInput")
    xT_d = nc.dram_tensor("xT", [KD, P, L], BF16, kind="ExternalInput")
    wv_d = nc.dram_tensor("wv", [KO // 4, P, KD, D], BF16, kind="ExternalInput")
    bvT_d = nc.dram_tensor("bvT", [P, KO], F32, kind="ExternalInput")
    fc_d = nc.dram_tensor("fc_w", [KO // 4, P, 4, D], BF16, kind="ExternalInput")
    fcb_d = nc.dram_tensor("fc_b", [1, D], F32, kind="ExternalInput")
    g_d = nc.dram_tensor("ln_g", [1, D], F32, kind="ExternalInput")
    b_d = nc.dram_tensor("ln_b", [1, D], F32, kind="ExternalInput")
    out_d = nc.dram_tensor("out", [L, D], F32, kind="ExternalOutput")

    x_v = x_d.ap()                                               # [2, P, 4, D]
    wv_v = wv_d.ap()                                             # [8, P, KD, D]
    fc_v = fc_d.ap()                                             # [8, P, 4, D]
    out_v = out_d.ap().rearrange("(t p) d -> t p d", p=P)        # [NT, P, D]

    with tile.TileContext(nc) as tc, ExitStack() as ctx:
        consts = ctx.enter_context(tc.tile_pool(name="consts", bufs=1))
        fcp = ctx.enter_context(tc.tile_pool(name="fcp", bufs=1))
        work = ctx.enter_context(tc.tile_pool(name="work", bufs=3))
        ypool = ctx.enter_context(tc.tile_pool(name="ypool", bufs=8))
        psum = ctx.enter_context(
            tc.tile_pool(name="psum", bufs=1, space=bass.MemorySpace.PSUM))

        # ---- constants -------------------------------------------------
        ones_col = consts.tile([P, 1], BF16)     # token-sum matmul rhs
        nc.gpsimd.memset(ones_col[:], 1.0)
        ones2 = consts.tile([2, P], F32)         # z+fc_b broadcast lhsT
        nc.gpsimd.memset(ones2[:], 1.0)
        eps_t = consts.tile([P, 1], F32)
        nc.gpsimd.memset(eps_t[:], EPS)
        ident_bf = consts.tile([1, 1], BF16)   # 1x1 identity for transposes
        nc.gpsimd.memset(ident_bf[:], 1.0)

        bvT_t = consts.tile([P, KO], F32)
        nc.sync.dma_start(bvT_t[:], bvT_d.ap())
        bv1024 = consts.tile([P, KO], F32)
        nc.scalar.mul(bv1024[:], bvT_t[:], float(L))

        fcb_t = consts.tile([1, D], F32)
        nc.sync.dma_start(fcb_t[:], fcb_d.ap())
        g_t = consts.tile([1, D], F32)
        nc.sync.dma_start(g_t[:], g_d.ap())
        b_t = consts.tile([1, D], F32)
        nc.sync.dma_start(b_t[:], b_d.ap())

        # broadcast ln_g / ln_b to all 128 partitions via a K=1 matmul
        g_bc = consts.tile([P, D], F32)
        b_bc = consts.tile([P, D], F32)
        for src, dst in ((g_t, g_bc), (b_t, b_bc)):
            pb = psum.tile([P, D], F32, tag="bcast")
            nc.tensor.matmul(pb[:], ones2[0:1, :], src[:], start=True, stop=True)
            nc.vector.tensor_copy(dst[:], pb[:])

        # ---- load x and x.T (stay resident in SBUF) --------------------
        x_t = consts.tile([P, NT, D], BF16)
        for u in range(2):
            nc.sync.dma_start(x_t[:, u * 4:(u + 1) * 4, :], x_v[u])
        xT_t = consts.tile([P, KD, L], BF16)
        for c in range(KD):
            nc.sync.dma_start(xT_t[:, c, :], xT_d.ap()[c])

        # ---- phase A: xsumT[d] = sum over tokens of x ------------------
        # lhsT = x tile slice [128 tok, 128 d], rhs = ones -> psum [128 d, 1]
        ps_xs = psum.tile([P, KD], F32, tag="xs")
        for c in range(KD):
            for t in range(NT):
                nc.tensor.matmul(
                    ps_xs[:, c:c + 1],
                    x_t[:, t, c * P:(c + 1) * P],
                    ones_col[:],
                    start=(t == 0), stop=(t == NT - 1))
        xsT = consts.tile([P, KD], BF16)
        nc.vector.tensor_copy(xsT[:], ps_xs[:])

        # ---- phase B+C interleaved, streamed by 512-wide hd blocks -----
        # block j: DMA wv cols [4j*128,(4j+4)*128) + fc rows likewise, then
        #   tT col o = sum_kd wv[:, kd, oc*128:..].T @ xsT[:, kd]   (psum)
        #   ACT copies col to SBUF adding L*bv, matmul2 accumulates z.
        ps_tT = psum.tile([P, KO], F32, tag="tT")
        tT_sb = consts.tile([P, KO], BF16)
        ps_z = psum.tile([1, D], F32, tag="z")
        for j in range(KO // 4):
            wv_bt = work.tile([P, KD, D], BF16, tag="wvb")
            nc.sync.dma_start(wv_bt[:], wv_v[j])
            fc_bt = work.tile([P, 4, D], BF16, tag="fcb")
            nc.sync.dma_start(fc_bt[:], fc_v[j])
            for oc in range(4):
                o = 4 * j + oc
                for kd in range(KD):
                    nc.tensor.matmul(
                        ps_tT[:, o:o + 1],
                        wv_bt[:, kd, oc * P:(oc + 1) * P],
                        xsT[:, kd:kd + 1],
                        start=(kd == 0), stop=(kd == KD - 1))
                nc.scalar.activation(tT_sb[:, o:o + 1], ps_tT[:, o:o + 1],
                                     AF.Identity, bias=bv1024[:, o:o + 1],
                                     scale=1.0)
                nc.tensor.matmul(
                    ps_z[:],
                    tT_sb[:, o:o + 1],
                    fc_bt[:, oc, :],
                    start=(o == 0), stop=(o == KO - 1),
                    skip_group_check=True)

        # ---- early layernorm stats on x (overlaps the z phase) ---------
        # Since z is constant across tokens:
        #   y = x + z',  z' = z + fc_b,  mz = mean(z'),  zc = z' - mz
        #   mean_y[t] = mean_x[t] + mz
        #   var_y[t]  = var_x[t] + (2/D) * (x_t . zc) + mean(zc^2)
        #   out[t]    = ((x_t - mean_x[t])*g + zc*g) * rstd[t] + b
        # so per-token x statistics and (x - mean_x)*g run during the z
        # phase on the otherwise-idle DVE, and the x.zc dots run on the
        # otherwise-idle PE afterwards.
        inv_d = 1.0 / D
        varx8 = consts.tile([P, NT], F32)
        xg_tiles = []
        for t in range(NT):
            s6 = work.tile([P, 6], F32, tag="s6")
            nc.vector.bn_stats(s6[:], x_t[:, t, :])
            mv = work.tile([P, 2], F32, tag="mv")
            nc.vector.bn_aggr(mv[:], s6[:])
            nc.vector.tensor_copy(varx8[:, t:t + 1], mv[:, 1:2])
            negmx = work.tile([P, 1], F32, tag="negmx")
            nc.vector.tensor_scalar_mul(negmx[:], mv[:, 0:1], -1.0)
            xg = ypool.tile([P, D], F32, tag="xg")
            nc.vector.scalar_tensor_tensor(
                xg[:], x_t[:, t, :], negmx[:], g_bc[:],
                op0=ALU.add, op1=ALU.mult)
            xg_tiles.append(xg)

        # ---- phase D: z tail -> zc, zg broadcast, variance pieces ------
        zrow = consts.tile([1, D], F32)
        nc.vector.tensor_add(zrow[:], fcb_t[:], ps_z[:])
        zsum = consts.tile([1, 1], F32)
        nc.vector.tensor_reduce(zsum[:], zrow[:], axis=mybir.AxisListType.X,
                                op=ALU.add)
        negmz = consts.tile([1, 1], F32)
        nc.scalar.mul(negmz[:], zsum[:], -inv_d)
        zc = consts.tile([1, D], F32)
        nc.scalar.activation(zc[:], zrow[:], AF.Identity, bias=negmz[:])
        zc_bf = consts.tile([1, D], BF16)
        nc.vector.tensor_copy(zc_bf[:], zc[:])

        # zcT (for the PE dot products)
        zcT = consts.tile([P, KD], BF16)
        for c in range(KD):
            pzt = psum.tile([P, 1], BF16, tag="zct")
            nc.tensor.transpose(pzt[:], zc_bf[0:1, c * P:(c + 1) * P],
                                ident_bf[0:1, 0:1])
            nc.vector.tensor_copy(zcT[:, c:c + 1], pzt[:])

        # zg = zc * g, broadcast to 128 partitions
        zg = consts.tile([1, D], F32)
        nc.vector.scalar_tensor_tensor(
            zg[:], zc[:], 1.0, g_t[:], op0=ALU.mult, op1=ALU.mult)
        ps_zg = psum.tile([P, D], F32, tag="bcast")
        nc.tensor.matmul(ps_zg[:], ones2[0:1, :], zg[:], start=True, stop=True)
        zg_bc = consts.tile([P, D], F32)
        nc.vector.tensor_copy(zg_bc[:], ps_zg[:])

        # mean(zc^2) broadcast into a per-partition bias (+ eps)
        zcsq = consts.tile([1, D], F32)
        nc.vector.tensor_mul(zcsq[:], zc[:], zc[:])
        ezsum = consts.tile([1, 1], F32)
        nc.vector.tensor_reduce(ezsum[:], zcsq[:], axis=mybir.AxisListType.X,
                                op=ALU.add)
        ps_ez = psum.tile([P, 1], F32, tag="ez")
        nc.tensor.matmul(ps_ez[:], ones2[0:1, :], ezsum[:],
                         start=True, stop=True)
        bias8 = consts.tile([P, 1], F32)
        nc.scalar.activation(bias8[:], ps_ez[:], AF.Identity,
                             bias=eps_t[:], scale=inv_d)

        # per-token dots x_t . zc on the PE
        dots8 = consts.tile([P, NT], F32)
        for t in range(NT):
            pd = psum.tile([P, 1], F32, tag="dots")
            for c in range(KD):
                nc.tensor.matmul(pd[:], xT_t[:, c, t * P:(t + 1) * P],
                                 zcT[:, c:c + 1],
                                 start=(c == 0), stop=(c == KD - 1))
            nc.vector.tensor_copy(dots8[:, t:t + 1], pd[:])

        var8 = consts.tile([P, NT], F32)
        nc.vector.scalar_tensor_tensor(
            var8[:], dots8[:], 2.0 * inv_d, varx8[:],
            op0=ALU.mult, op1=ALU.add)
        std8 = consts.tile([P, NT], F32)
        nc.scalar.activation(std8[:], var8[:], AF.Sqrt, bias=bias8[:])
        rstd8 = consts.tile([P, NT], F32)
        nc.vector.reciprocal(rstd8[:], std8[:])

        # ---- phase E: two DVE passes per tile + store ------------------
        for t in range(NT):
            yg = work.tile([P, D], F32, tag="yg", bufs=4)
            nc.vector.tensor_add(yg[:], xg_tiles[t][:], zg_bc[:])
            o2 = work.tile([P, D], F32, tag="o2", bufs=4)
            nc.vector.scalar_tensor_tensor(
                o2[:], yg[:], rstd8[:, t:t + 1], b_bc[:],
                op0=ALU.mult, op1=ALU.add)
            nc.sync.dma_start(out_v[t], o2[:])

    nc.compile()
    return nc


_NC_CACHE = None


def _get_nc():
    global _NC_CACHE
    if _NC_CACHE is None:
        _NC_CACHE = build_kernel()
    return _NC_CACHE


def _shard_inputs(inputs):
    bf = ml_dtypes.bfloat16
    x = np.asarray(inputs["input"], dtype=np.float32)
    wv = np.asarray(inputs["wv"], dtype=np.float32)
    bv = np.asarray(inputs["bv"], dtype=np.float32)
    fc_w = np.asarray(inputs["fc_w"], dtype=np.float32)
    fc_b = np.asarray(inputs["fc_b"], dtype=np.float32)
    ln_g = np.asarray(inputs["ln_g"], dtype=np.float32)
    ln_b = np.asarray(inputs["ln_b"], dtype=np.float32)

    # blocked layouts (see build_kernel): each 1MB DMA is contiguous
    wv_bl = np.ascontiguousarray(
        wv.reshape(KD, P, KO // 4, D).transpose(2, 1, 0, 3)).astype(bf)
    fc_bl = np.ascontiguousarray(
        fc_w.reshape(KO // 4, 4, P, D).transpose(0, 2, 1, 3)).astype(bf)
    bvT = np.ascontiguousarray(bv.reshape(KO, P).T)          # [128, 32]
    fcb = np.ascontiguousarray(fc_b[None, :])
    g = np.ascontiguousarray(ln_g[None, :])
    b = np.ascontiguousarray(ln_b[None, :])

    in_maps = []
    for i in range(N_CORES):
        x_bl = np.ascontiguousarray(
            x[i].reshape(2, 4, P, D).transpose(0, 2, 1, 3)).astype(bf)
        xT_bl = np.ascontiguousarray(x[i].T.reshape(KD, P, L)).astype(bf)
        in_maps.append({
            "x": x_bl,
            "xT": xT_bl,
            "wv": wv_bl,
            "bvT": bvT,
            "fc_w": fc_bl,
            "fc_b": fcb,
            "ln_g": g,
            "ln_b": b,
        })
    return in_maps


def kernel(**inputs) -> np.ndarray:
    nc = _get_nc()
    in_maps = _shard_inputs(inputs)
    res = run_bass_kernel_spmd(nc, in_maps, core_ids=list(range(N_CORES)))
    out = np.stack([res.results[i]["out"] for i in range(N_CORES)], axis=0)
    return out.astype(np.float32)


def _install_ntff_hook_shim():
    """Bridge trn_boot's ctypes NTFF profiler into antenv.axon_hooks,
    which bass_utils imports when trace=True under axon."""
    import sys
    import types
    try:
        from antenv.axon_hooks import get_axon_ntff_profile_hook  # noqa: F401
        return
    except ImportError:
        pass
    try:
        from trn_agent_boot.trn_boot import _ntff_profile_via_ctypes
        hook = _ntff_profile_via_ctypes("/opt/axon/libaxon_pjrt.so")
    except Exception:
        hook = None
    mod = types.ModuleType("antenv.axon_hooks")
    state = {"hook": hook}
    mod.get_axon_ntff_profile_hook = lambda: state["hook"]
    mod.set_axon_ntff_profile_hook = lambda h: state.update(hook=h)
    sys.modules["antenv.axon_hooks"] = mod
    import antenv
    antenv.axon_hooks = mod


def kernel_profiled(inputs, trace_cores=None):
    """Like kernel() but with trace=True; returns (out, BassKernelResults)."""
    _install_ntff_hook_shim()
    nc = _get_nc()
    in_maps = _shard_inputs(inputs)
    res = run_bass_kernel_spmd(
        nc, in_maps, core_ids=list(range(N_CORES)), trace=True,
        trace_cores=trace_cores if trace_cores is not None else [0])
    out = np.stack([res.results[i]["out"] for i in range(N_CORES)], axis=0)
    return out.astype(np.float32), res


if __name__ == "__main__":
    import sys
    if "--sim" in sys.argv:
        # quick single-core CoreSim check against the collapsed math
        from concourse.bass_interp import CoreSim
        rng = np.random.default_rng(0)
        x = rng.standard_normal((B, L, D), dtype=np.float32)
        wv = rng.standard_normal((D, HD), dtype=np.float32) * 0.04
        bv = rng.standard_normal(HD, dtype=np.float32) * 0.04
        fc_w = rng.standard_normal((HD, D), dtype=np.float32) * 0.015
        fc_b = rng.standard_normal(D, dtype=np.float32) * 0.015
        g = rng.standard_normal(D, dtype=np.float32) * 0.3 + 1.0
        b = rng.standard_normal(D, dtype=np.float32) * 0.1
        inputs = dict(input=x, wv=wv, bv=bv, fc_w=fc_w, fc_b=fc_b,
                      ln_g=g, ln_b=b)

        nc = _get_nc()
        in_maps = _shard_inputs(inputs)
        sim = CoreSim(nc, trace=False)
        for k, v in in_maps[0].items():
            sim.tensor(k)[:] = v
        sim.simulate()
        got = np.array(sim.tensor("out"))

        xsum = x[0].sum(0)
        z = (xsum @ wv + L * bv) @ fc_w + fc_b
        y = x[0] + z[None, :]
        mu = y.mean(-1, keepdims=True)
        var = y.var(-1, keepdims=True)
        want = (y - mu) / np.sqrt(var + EPS) * g + b
        err = np.abs(got - want).max() / np.abs(want).max()
        print("sim absmax rel err:", err)
        assert err < 2e-2, err
        print("SIM PASS")


# revision 28
# speedup vs baseline: 2.2577x; 1.2036x over previous
"""Trainium2 Bass kernel for nn_MultiHeadAttention_26482768347194.

Key algebraic fact: the reference applies softmax over a size-1 trailing
axis, so the attention score matrix is exactly all-ones.  The whole module
collapses (exactly, in real arithmetic) to

    xsum[b]   = sum_l x[b, l, :]                        # (D,)
    t[b]      = xsum[b] @ wv + L * bv                   # (H*D,)
    z[b]      = t[b] @ fc_w + fc_b                      # (D,)
    y[b,l,:]  = x[b,l,:] + z[b]
    out       = LayerNorm(y) * ln_g + ln_b              # over last dim

q/k/tanh/score inputs are mathematically dead.

Sharding (V3): pure data-parallel over batch, one batch element per core,
weights replicated.  Cross-core collectives measured ~70us under this
runtime (launch-skew barrier), so each core runs fully independently:
  1. xsum.T from its x shard (x-stationary PE matmuls against ones),
  2. t.T = wv.T @ xsum.T per 128-wide hd column (wv chunks stationary),
  3. z = t @ fc_w (t.T columns stationary, fc rows moving), + fc_b,
  4. broadcast z to 128 partitions via a K=2 ones-matmul (adds fc_b),
  5. residual + LayerNorm with batched per-tile statistics.

PE matmuls run in bf16 (fp32 is 2-4x slower on the PE and doubles DMA);
the residual/LN statistics stay fp32.  End-to-end absmax relative error
vs the fp32 reference: ~2e-3 (bf16 weight rounding, well inside the
scale-relative gate).

This file is self-contained: shapes are hardcoded, no sibling imports.
"""

from contextlib import ExitStack

import numpy as np
import ml_dtypes

import concourse.bass as bass
import concourse.bacc as bacc
import concourse.mybir as mybir
import concourse.tile as tile
from concourse.bass_utils import run_bass_kernel_spmd

B, L, D, H = 8, 1024, 512, 8
HD = H * D          # 4096
P = 128             # partitions
NT = L // P         # 8 token tiles per core
KD = D // P         # 4 contraction chunks over d
KO = HD // P        # 32 contraction chunks over h*d
EPS = 1e-5
N_CORES = 8

F32 = mybir.dt.float32
BF16 = mybir.dt.bfloat16
AF = mybir.ActivationFunctionType
ALU = mybir.AluOpType

# which engine runs the final "+ ln_b" pass (see LN phase).  GpSimd shares
# an SBUF port with the DVE (exclusive lock), so offloading there slows
# BOTH engines to ~1.7us per pass — keep it on the DVE.
LN_B_ON_GPSIMD = False


def build_kernel():
    nc = bacc.Bacc("TRN2", target_bir_lowering=False, debug=False,
                   num_devices=N_CORES)

    # x / wv / fc arrive pre-blocked from the host so that every big DMA
    # reads a fully contiguous region:
    #   x[u, p, t, d]  = x_orig[(u*4 + t)*128 + p, d]          (2 x 1MB)
    #   wv[j, p, k, c] = wv_orig[k*128 + p, j*512 + c]         (8 x 1MB)
    #   fc[j, p, q, d] = fc_orig[(j*4 + q)*128 + p, d]         (8 x 1MB)
    #   xT[c, p, l]    = x_orig[l, c*128 + p]                  (4 x 256KB)
    x_d = nc.dram_tensor("x", [2, P, 4, D], BF16, kind="ExternalInput")
    xT_d = nc.dram_tensor("xT", [KD, P, L], BF16, kind="ExternalInput")
    wv_d = nc.dram_tensor("wv", [KO // 4, P, KD, D], BF16, kind="ExternalInput")
    bvT_d = nc.dram_tensor("bvT", [P, KO], F32, kind="ExternalInput")
    fc_d = nc.dram_tensor("fc_w", [KO // 4, P, 4, D], BF16, kind="ExternalInput")
    fcb_d = nc.dram_tensor("fc_b", [1, D], F32, kind="ExternalInput")
    g_d = nc.dram_tensor("ln_g", [1, D], F32, kind="ExternalInput")
    b_d = nc.dram_tensor("ln_b", [1, D], F32, kind="ExternalInput")
    out_d = nc.dram_tensor("out", [L, D], F32, kind="ExternalOutput")

    x_v = x_d.ap()                                               # [2, P, 4, D]
    wv_v = wv_d.ap()                                             # [8, P, KD, D]
    fc_v = fc_d.ap()                                             # [8, P, 4, D]
    out_v = out_d.ap().rearrange("(t p) d -> t p d", p=P)        # [NT, P, D]

    with tile.TileContext(nc) as tc, ExitStack() as ctx:
        consts = ctx.enter_context(tc.tile_pool(name="consts", bufs=1))
        fcp = ctx.enter_context(tc.tile_pool(name="fcp", bufs=1))
        work = ctx.enter_context(tc.tile_pool(name="work", bufs=3))
        ypool = ctx.enter_context(tc.tile_pool(name="ypool", bufs=8))
        psum = ctx.enter_context(
            tc.tile_pool(name="psum", bufs=1, space=bass.MemorySpace.PSUM))

        # ---- constants -------------------------------------------------
        ones_col = consts.tile([P, 1], BF16)     # token-sum matmul rhs
        nc.gpsimd.memset(ones_col[:], 1.0)
        ones2 = consts.tile([2, P], F32)         # z+fc_b broadcast lhsT
        nc.gpsimd.memset(ones2[:], 1.0)
        eps_t = consts.tile([P, 1], F32)
        nc.gpsimd.memset(eps_t[:], EPS)
        ident_bf = consts.tile([1, 1], BF16)   # 1x1 identity for transposes
        nc.gpsimd.memset(ident_bf[:], 1.0)

        bvT_t = consts.tile([P, KO], F32)
        nc.sync.dma_start(bvT_t[:], bvT_d.ap())
        bv1024 = consts.tile([P, KO], F32)
        nc.scalar.mul(bv1024[:], bvT_t[:], float(L))

        fcb_t = consts.tile([1, D], F32)
        nc.sync.dma_start(fcb_t[:], fcb_d.ap())
        g_t = consts.tile([1, D], F32)
        nc.sync.dma_start(g_t[:], g_d.ap())
        b_t = consts.tile([1, D], F32)
        nc.sync.dma_start(b_t[:], b_d.ap())

        # broadcast ln_g / ln_b to all 128 partitions via a K=1 matmul
        g_bc = consts.tile([P, D], F32)
        b_bc = consts.tile([P, D], F32)
        for src, dst in ((g_t, g_bc), (b_t, b_bc)):
            pb = psum.tile([P, D], F32, tag="bcast")
            nc.tensor.matmul(pb[:], ones2[0:1, :], src[:], start=True, stop=True)
            nc.vector.tensor_copy(dst[:], pb[:])

        # ---- load x and x.T (stay resident in SBUF) --------------------
        x_t = consts.tile([P, NT, D], BF16)
        for u in range(2):
            nc.sync.dma_start(x_t[:, u * 4:(u + 1) * 4, :], x_v[u])
        xT_t = consts.tile([P, KD, L], BF16)
        for c in range(KD):
            nc.sync.dma_start(xT_t[:, c, :], xT_d.ap()[c])

        # ---- phase A: xsumT[d] = sum over tokens of x ------------------
        # lhsT = x tile slice [128 tok, 128 d], rhs = ones -> psum [128 d, 1]
        ps_xs = psum.tile([P, KD], F32, tag="xs")
        for c in range(KD):
            for t in range(NT):
                nc.tensor.matmul(
                    ps_xs[:, c:c + 1],
                    x_t[:, t, c * P:(c + 1) * P],
                    ones_col[:],
                    start=(t == 0), stop=(t == NT - 1))
        xsT = consts.tile([P, KD], BF16)
        nc.vector.tensor_copy(xsT[:], ps_xs[:])

        # ---- phase B+C interleaved, streamed by 512-wide hd blocks -----
        # block j: DMA wv cols [4j*128,(4j+4)*128) + fc rows likewise, then
        #   tT col o = sum_kd wv[:, kd, oc*128:..].T @ xsT[:, kd]   (psum)
        #   ACT copies col to SBUF adding L*bv, matmul2 accumulates z.
        tT_sb = consts.tile([P, KO], BF16)
        ps_z = psum.tile([1, D], F32, tag="z")
        for j in range(KO // 4):
            wv_bt = work.tile([P, KD, D], BF16, tag="wvb", bufs=4)
            nc.sync.dma_start(wv_bt[:], wv_v[j])
            fc_bt = work.tile([P, 4, D], BF16, tag="fcb", bufs=4)
            nc.sync.dma_start(fc_bt[:], fc_v[j])
            for oc in range(4):
                o = 4 * j + oc
                # per-column psum tile (rotating banks) so the ACT read of
                # column o doesn't serialize the PE writes of column o+1
                ps_col = psum.tile([P, 1], F32, tag="small", bufs=4)
                for kd in range(KD):
                    nc.tensor.matmul(
                        ps_col[:],
                        wv_bt[:, kd, oc * P:(oc + 1) * P],
                        xsT[:, kd:kd + 1],
                        start=(kd == 0), stop=(kd == KD - 1))
                nc.scalar.activation(tT_sb[:, o:o + 1], ps_col[:],
                                     AF.Identity, bias=bv1024[:, o:o + 1],
                                     scale=1.0)
                nc.tensor.matmul(
                    ps_z[:],
                    tT_sb[:, o:o + 1],
                    fc_bt[:, oc, :],
                    start=(o == 0), stop=(o == KO - 1),
                    skip_group_check=True)

        # ---- early layernorm stats on x (overlaps the z phase) ---------
        # Since z is constant across tokens:
        #   y = x + z',  z' = z + fc_b,  mz = mean(z'),  zc = z' - mz
        #   mean_y[t] = mean_x[t] + mz
        #   var_y[t]  = var_x[t] + (2/D) * (x_t . zc) + mean(zc^2)
        #   out[t]    = ((x_t - mean_x[t])*g + zc*g) * rstd[t] + b
        # so per-token x statistics and (x - mean_x)*g run during the z
        # phase on the otherwise-idle DVE, and the x.zc dots run on the
        # otherwise-idle PE afterwards.
        inv_d = 1.0 / D
        varx8 = consts.tile([P, NT], F32)
        xg_tiles = []
        for t in range(NT):
            s6 = work.tile([P, 6], F32, tag="s6")
            nc.vector.bn_stats(s6[:], x_t[:, t, :])
            mv = work.tile([P, 2], F32, tag="mv")
            nc.vector.bn_aggr(mv[:], s6[:])
            nc.vector.tensor_copy(varx8[:, t:t + 1], mv[:, 1:2])
            negmx = work.tile([P, 1], F32, tag="negmx")
            nc.vector.tensor_scalar_mul(negmx[:], mv[:, 0:1], -1.0)
            xg = ypool.tile([P, D], F32, tag="xg")
            nc.vector.scalar_tensor_tensor(
                xg[:], x_t[:, t, :], negmx[:], g_bc[:],
                op0=ALU.add, op1=ALU.mult)
            xg_tiles.append(xg)

        # ---- phase D: z tail -> zc, zg broadcast, variance pieces ------
        zrow = consts.tile([1, D], F32)
        nc.vector.tensor_add(zrow[:], fcb_t[:], ps_z[:])
        zsum = consts.tile([1, 1], F32)
        nc.vector.tensor_reduce(zsum[:], zrow[:], axis=mybir.AxisListType.X,
                                op=ALU.add)
        negmz = consts.tile([1, 1], F32)
        nc.scalar.mul(negmz[:], zsum[:], -inv_d)
        zc = consts.tile([1, D], F32)
        nc.scalar.activation(zc[:], zrow[:], AF.Identity, bias=negmz[:])
        zc_bf = consts.tile([1, D], BF16)
        nc.vector.tensor_copy(zc_bf[:], zc[:])

        # zcT (for the PE dot products)
        zcT = consts.tile([P, KD], BF16)
        for c in range(KD):
            pzt = psum.tile([P, 1], BF16, tag="small", bufs=4)
            nc.tensor.transpose(pzt[:], zc_bf[0:1, c * P:(c + 1) * P],
                                ident_bf[0:1, 0:1])
            nc.vector.tensor_copy(zcT[:, c:c + 1], pzt[:])

        # zg = zc * g, broadcast to 128 partitions
        zg = consts.tile([1, D], F32)
        nc.vector.scalar_tensor_tensor(
            zg[:], zc[:], 1.0, g_t[:], op0=ALU.mult, op1=ALU.mult)
        ps_zg = psum.tile([P, D], F32, tag="bcast")
        nc.tensor.matmul(ps_zg[:], ones2[0:1, :], zg[:], start=True, stop=True)
        zg_bc = consts.tile([P, D], F32)
        nc.vector.tensor_copy(zg_bc[:], ps_zg[:])

        # mean(zc^2) broadcast into a per-partition bias (+ eps)
        zcsq = consts.tile([1, D], F32)
        nc.vector.tensor_mul(zcsq[:], zc[:], zc[:])
        ezsum = consts.tile([1, 1], F32)
        nc.vector.tensor_reduce(ezsum[:], zcsq[:], axis=mybir.AxisListType.X,
                                op=ALU.add)
        ps_ez = psum.tile([P, 1], F32, tag="small", bufs=4)
        nc.tensor.matmul(ps_ez[:], ones2[0:1, :], ezsum[:],
                         start=True, stop=True)
        bias8 = consts.tile([P, 1], F32)
        nc.scalar.activation(bias8[:], ps_ez[:], AF.Identity,
                             bias=eps_t[:], scale=inv_d)

        # ---- phase E: per-token dots on the PE, then two DVE passes ----
        for t in range(NT):
            pd = psum.tile([P, 1], F32, tag="small", bufs=4)
            for c in range(KD):
                nc.tensor.matmul(pd[:], xT_t[:, c, t * P:(t + 1) * P],
                                 zcT[:, c:c + 1],
                                 start=(c == 0), stop=(c == KD - 1))
            var_t = work.tile([P, 1], F32, tag="var_t", bufs=4)
            nc.vector.scalar_tensor_tensor(
                var_t[:], pd[:], 2.0 * inv_d, varx8[:, t:t + 1],
                op0=ALU.mult, op1=ALU.add)
            std_t = work.tile([P, 1], F32, tag="std_t", bufs=4)
            nc.scalar.activation(std_t[:], var_t[:], AF.Sqrt, bias=bias8[:])
            rstd_t = work.tile([P, 1], F32, tag="rstd_t", bufs=4)
            nc.vector.reciprocal(rstd_t[:], std_t[:])

            yg = work.tile([P, D], F32, tag="yg", bufs=4)
            nc.vector.tensor_add(yg[:], xg_tiles[t][:], zg_bc[:])
            o2 = work.tile([P, D], F32, tag="o2", bufs=4)
            nc.vector.scalar_tensor_tensor(
                o2[:], yg[:], rstd_t[:], b_bc[:],
                op0=ALU.mult, op1=ALU.add)
            nc.sync.dma_start(out_v[t], o2[:])

    nc.compile()
    return nc


_NC_CACHE = None


def _get_nc():
    global _NC_CACHE
    if _NC_CACHE is None:
        _NC_CACHE = build_kernel()
    return _NC_CACHE


def _shard_inputs(inputs):
    bf = ml_dtypes.bfloat16
    x = np.asarray(inputs["input"], dtype=np.float32)
    wv = np.asarray(inputs["wv"], dtype=np.float32)
    bv = np.asarray(inputs["bv"], dtype=np.float32)
    fc_w = np.asarray(inputs["fc_w"], dtype=np.float32)
    fc_b = np.asarray(inputs["fc_b"], dtype=np.float32)
    ln_g = np.asarray(inputs["ln_g"], dtype=np.float32)
    ln_b = np.asarray(inputs["ln_b"], dtype=np.float32)

    # blocked layouts (see build_kernel): each 1MB DMA is contiguous
    wv_bl = np.ascontiguousarray(
        wv.reshape(KD, P, KO // 4, D).transpose(2, 1, 0, 3)).astype(bf)
    fc_bl = np.ascontiguousarray(
        fc_w.reshape(KO // 4, 4, P, D).transpose(0, 2, 1, 3)).astype(bf)
    bvT = np.ascontiguousarray(bv.reshape(KO, P).T)          # [128, 32]
    fcb = np.ascontiguousarray(fc_b[None, :])
    g = np.ascontiguousarray(ln_g[None, :])
    b = np.ascontiguousarray(ln_b[None, :])

    in_maps = []
    for i in range(N_CORES):
        x_bl = np.ascontiguousarray(
            x[i].reshape(2, 4, P, D).transpose(0, 2, 1, 3)).astype(bf)
        xT_bl = np.ascontiguousarray(x[i].T.reshape(KD, P, L)).astype(bf)
        in_maps.append({
            "x": x_bl,
            "xT": xT_bl,
            "wv": wv_bl,
            "bvT": bvT,
            "fc_w": fc_bl,
            "fc_b": fcb,
            "ln_g": g,
            "ln_b": b,
        })
    return in_maps


def kernel(**inputs) -> np.ndarray:
    nc = _get_nc()
    in_maps = _shard_inputs(inputs)
    res = run_bass_kernel_spmd(nc, in_maps, core_ids=list(range(N_CORES)))
    out = np.stack([res.results[i]["out"] for i in range(N_CORES)], axis=0)
    return out.astype(np.float32)


def _install_ntff_hook_shim():
    """Bridge trn_boot's ctypes NTFF profiler into antenv.axon_hooks,
    which bass_utils imports when trace=True under axon."""
    import sys
    import types
    try:
        from antenv.axon_hooks import get_axon_ntff_profile_hook  # noqa: F401
        return
    except ImportError:
        pass
    try:
        from trn_agent_boot.trn_boot import _ntff_profile_via_ctypes
        hook = _ntff_profile_via_ctypes("/opt/axon/libaxon_pjrt.so")
    except Exception:
        hook = None
    mod = types.ModuleType("antenv.axon_hooks")
    state = {"hook": hook}
    mod.get_axon_ntff_profile_hook = lambda: state["hook"]
    mod.set_axon_ntff_profile_hook = lambda h: state.update(hook=h)
    sys.modules["antenv.axon_hooks"] = mod
    import antenv
    antenv.axon_hooks = mod


def kernel_profiled(inputs, trace_cores=None):
    """Like kernel() but with trace=True; returns (out, BassKernelResults)."""
    _install_ntff_hook_shim()
    nc = _get_nc()
    in_maps = _shard_inputs(inputs)
    res = run_bass_kernel_spmd(
        nc, in_maps, core_ids=list(range(N_CORES)), trace=True,
        trace_cores=trace_cores if trace_cores is not None else [0])
    out = np.stack([res.results[i]["out"] for i in range(N_CORES)], axis=0)
    return out.astype(np.float32), res


if __name__ == "__main__":
    import sys
    if "--sim" in sys.argv:
        # quick single-core CoreSim check against the collapsed math
        from concourse.bass_interp import CoreSim
        rng = np.random.default_rng(0)
        x = rng.standard_normal((B, L, D), dtype=np.float32)
        wv = rng.standard_normal((D, HD), dtype=np.float32) * 0.04
        bv = rng.standard_normal(HD, dtype=np.float32) * 0.04
        fc_w = rng.standard_normal((HD, D), dtype=np.float32) * 0.015
        fc_b = rng.standard_normal(D, dtype=np.float32) * 0.015
        g = rng.standard_normal(D, dtype=np.float32) * 0.3 + 1.0
        b = rng.standard_normal(D, dtype=np.float32) * 0.1
        inputs = dict(input=x, wv=wv, bv=bv, fc_w=fc_w, fc_b=fc_b,
                      ln_g=g, ln_b=b)

        nc = _get_nc()
        in_maps = _shard_inputs(inputs)
        sim = CoreSim(nc, trace=False)
        for k, v in in_maps[0].items():
            sim.tensor(k)[:] = v
        sim.simulate()
        got = np.array(sim.tensor("out"))

        xsum = x[0].sum(0)
        z = (xsum @ wv + L * bv) @ fc_w + fc_b
        y = x[0] + z[None, :]
        mu = y.mean(-1, keepdims=True)
        var = y.var(-1, keepdims=True)
        want = (y - mu) / np.sqrt(var + EPS) * g + b
        err = np.abs(got - want).max() / np.abs(want).max()
        print("sim absmax rel err:", err)
        assert err < 2e-2, err
        print("SIM PASS")
